# revision 30
# baseline (speedup 1.0000x reference)
"""Trainium2 Bass kernel for nn_CellNetwork (GNN + CWNN message passing).

Self-contained: takes FULL inputs, shards across 8 NeuronCores internally,
returns FULL output [20000, 256] fp32.

Strategy (SPMD, one program for all 8 ranks; per-rank data differs but all
instruction shapes are rank-uniform via max-over-ranks padding):
  - fp16 on device, fp32 PSUM accumulation.
  - Host->device traffic is minimized: only per-rank feature shards and
    compact int16 token indices ship. Replicated gather tables are built on
    device via AllGather; one-hot S matrices are built on the fly from 2-byte
    row indices (iota compare on DVE); accumulators are zeroed on device.
  - GNN over nodes: sharded by node (2500/rank). Segment-sum via "S-matrix"
    PE matmuls on a fixed grid of 128-row windows; h tables re-replicated
    per layer via AllGather.
  - CWNN over cells: sharded by cell (25000/rank). Messages gathered from a
    replicated he table (dma_gather, int16 indices -> 7 x 32768-row buckets),
    scaled by val (DVE broadcast mult), scatter-added (dma_scatter_add) into
    local accumulators; duplicate-row races avoided by splitting tokens into
    "rounds" with unique rows, serialized by Tile's WAW tracking.
  - Final dual scatter-add of cells to nodes: grid-S matmuls over he3.
"""
import sys
import numpy as np

sys.path.insert(0, "/opt/trn_rl_repo")

N = 20000
E = 200000
D = 128
NNZ = 400000
L = 3
NCORES = 8
RN = N // NCORES          # 2500 nodes per rank
RE = E // NCORES          # 25000 cells per rank
BUCK = 32768              # gather index window (int16 limit)
NBUCK_E = (NCORES * 25088 + BUCK - 1) // BUCK   # 7 (on padded table)
GWIN = 128                # grid-S window (rows per PSUM tile)
NWIN_N = (RN + GWIN - 1) // GWIN   # 20 windows for 2500 local nodes
RNP = NWIN_N * GWIN       # 2560 padded local nodes
WWIN = 512                # W-phase window (cols per matmul)
MAXTOK = 1024             # max tokens per dma_gather/scatter (SWDGE ring = 1024 descs)
REP = 25088               # cells per rank, padded to 49*512
EP_TBL = NCORES * REP     # 200704-row padded cell table
NP_TBL = NCORES * RNP     # 20480-row padded node table


def _pad128(n):
    return (n + 127) & ~127


def _ceil(a, b):
    return (a + b - 1) // b


# ---------------------------------------------------------------------------
# Host-side preprocessing
# ---------------------------------------------------------------------------

def _idx16(tokens, n_slots):
    """[16, n_slots/16] int16: token t at (t%16, t//16). Expanded to 128
    partitions (x8 replication) on device."""
    W = n_slots // 16
    arr = np.zeros((16, W), np.int16)
    t = np.arange(len(tokens))
    arr[t % 16, t // 16] = tokens.astype(np.int16)
    return arr


def _val_tile(vals, n_slots):
    """[128, n_slots/128] fp16: token t at (t%128, t//128)."""
    G = n_slots // 128
    arr = np.zeros((128, G), np.float16)
    t = np.arange(len(vals))
    arr[t % 128, t // 128] = vals.astype(np.float16)
    return arr


def _row_tile(rows, n_slots):
    """[128, n_slots/128] fp16: window-row of token t at (t%128, t//128);
    padded slots get -1 (matches no iota value -> zero S column)."""
    G = n_slots // 128
    arr = np.full((128, G), -1.0, np.float16)
    t = np.arange(len(rows))
    arr[t % 128, t // 128] = rows.astype(np.float16)
    return arr


def _prep_grid_s(per_rank_tokens, n_cells_keys):
    """Grid-S packing: tokens grouped into cells (ordered by key tuple list
    n_cells_keys); each cell padded to max-over-ranks chunk count."""
    nr = len(per_rank_tokens)
    cell_slots = []
    for key in n_cells_keys:
        mx = 0
        for r in range(nr):
            t = per_rank_tokens[r].get(key)
            if t is not None:
                mx = max(mx, len(t[0]))
        cell_slots.append(_pad128(mx))
    total = int(np.sum(cell_slots))
    offs = np.zeros(len(cell_slots) + 1, np.int64)
    offs[1:] = np.cumsum(cell_slots)
    return cell_slots, offs, total


class _Prep:
    pass


def _preprocess(x, xe, gnn_W, cwnn_W0, cwnn_W1, cwnn_W2, Ldo_val, Lup_val,
                edges, row, col, Ldo_idx, Lup_idx):
    p = _Prep()
    f16 = np.float16
    x16 = np.asarray(x, np.float32).astype(f16)
    xe16 = np.asarray(xe, np.float32).astype(f16)
    # per-rank padded shards
    p.x_sh = []
    p.xe_sh = []
    for r in range(NCORES):
        xs = np.zeros((RNP, D), f16)
        xs[:RN] = x16[r * RN:(r + 1) * RN]
        p.x_sh.append(xs)
        xes = np.zeros((REP, D), f16)
        xes[:RE] = xe16[r * RE:(r + 1) * RE]
        p.xe_sh.append(xes)
    p.gnnW = np.ascontiguousarray(
        np.asarray(gnn_W, np.float32).astype(f16).transpose(1, 0, 2).reshape(D, L * D))
    p.W0 = np.ascontiguousarray(
        np.asarray(cwnn_W0, np.float32).astype(f16).transpose(1, 0, 2).reshape(D, L * D))
    p.W1 = np.ascontiguousarray(
        np.asarray(cwnn_W1, np.float32).astype(f16).transpose(1, 0, 2).reshape(D, L * D))
    p.W2 = np.ascontiguousarray(
        np.asarray(cwnn_W2, np.float32).astype(f16).transpose(1, 0, 2).reshape(D, L * D))
    p.ident = np.eye(D, dtype=f16)
    p.iota = np.tile(np.arange(128, dtype=f16)[None, :], (128, 1))
    p.iota256 = np.tile(np.arange(256, dtype=f16)[None, :], (128, 1))

    edges = np.asarray(edges, np.int64)
    row = np.asarray(row, np.int64)
    col = np.asarray(col, np.int64)
    Ldo_idx = np.asarray(Ldo_idx, np.int64)
    Lup_idx = np.asarray(Lup_idx, np.int64)

    # ---------------- GNN tokens (grid-S, 1 bucket, 20 windows) -----------
    src, dst = edges[0], edges[1]
    rk = dst // RN
    per_rank = []
    for r in range(NCORES):
        sel = rk == r
        s_src = src[sel]
        s_src = (s_src // RN) * RNP + s_src % RN
        s_row = dst[sel] - r * RN
        w = s_row // GWIN
        order = np.argsort(w, kind="stable")
        s_src, s_row, w = s_src[order], s_row[order], w[order]
        d = {}
        for wi in range(NWIN_N):
            m = w == wi
            d[wi] = (s_src[m], s_row[m], None)
        per_rank.append(d)
    keys = list(range(NWIN_N))
    cs, offs, Tg = _prep_grid_s(per_rank, keys)
    p.gnn_win_slots = cs          # slots per window (multiple of 128)
    p.gnn_Tg = Tg
    p.gnn_idx = []
    p.gnn_row = []
    for r in range(NCORES):
        toks = np.zeros(Tg, np.int64)
        rows_all = np.full(Tg, -1, np.int64)
        for wi in range(NWIN_N):
            idx, rows, _ = per_rank[r][wi]
            o = offs[wi]
            n = len(idx)
            toks[o:o + n] = idx
            rows_all[o:o + n] = rows - wi * GWIN
        p.gnn_idx.append(_idx16(toks, Tg))
        p.gnn_row.append(_row_tile(rows_all, Tg))

    # ---------------- CWNN tokens (lap x bucket x 256-row window grid) -----
    # token: (col_global, row_local, val) ; lap 0 = Ldo, 1 = Lup
    # Receive side is S-matmul into SBUF agg (no dma_scatter_add): tokens
    # sorted by (lap, bucket, window); each cell padded to max-over-ranks
    # (multiple of 128); duplicates within a chunk are fine (PE sums them).
    lap_data = [(Ldo_idx, np.asarray(Ldo_val)), (Lup_idx, np.asarray(Lup_val))]
    CWWIN = 256
    NWIN_E = REP // CWWIN     # 98
    cell_tokens = [dict() for _ in range(NCORES)]  # key (lap, b, w)
    for lap, (lidx, lval) in enumerate(lap_data):
        lrow, lcol = lidx[0], lidx[1]
        rr = lrow // RE
        for r in range(NCORES):
            sel = rr == r
            c_g = lcol[sel]
            c_g = (c_g // RE) * REP + c_g % RE
            r_l = lrow[sel] - r * RE
            v = lval[sel]
            b = c_g // BUCK
            w = r_l // CWWIN
            key = b * NWIN_E + w
            order = np.argsort(key, kind="stable")
            c_g, r_l, v, key = c_g[order], r_l[order], v[order], key[order]
            for bb in range(NBUCK_E):
                for ww in range(NWIN_E):
                    m = key == bb * NWIN_E + ww
                    if m.any():
                        cell_tokens[r][(lap, bb, ww)] = (
                            c_g[m] - bb * BUCK, r_l[m] - ww * CWWIN, v[m])
    cw_keys = [(lap, bb, ww) for lap in range(2) for bb in range(NBUCK_E)
               for ww in range(NWIN_E)]
    cw_slots = []
    for key in cw_keys:
        mx = 0
        for r in range(NCORES):
            t = cell_tokens[r].get(key)
            if t is not None:
                mx = max(mx, len(t[0]))
        cw_slots.append(_pad128(mx))
    Tc = int(np.sum(cw_slots))
    coffs = np.zeros(len(cw_slots) + 1, np.int64)
    coffs[1:] = np.cumsum(cw_slots)
    p.cw_keys = cw_keys
    p.cw_slots = cw_slots
    p.cw_offs = coffs
    p.cw_Tc = Tc
    p.CWWIN = CWWIN
    p.cw_gidx = []
    p.cw_row = []
    p.cw_val = []
    for r in range(NCORES):
        gt = np.zeros(Tc, np.int64)
        rt = np.full(Tc, -1, np.int64)
        vt = np.zeros(Tc, np.float32)
        for si, key in enumerate(cw_keys):
            t = cell_tokens[r].get(key)
            if t is None:
                continue
            o = coffs[si]
            n = len(t[0])
            gt[o:o + n] = t[0]
            rt[o:o + n] = t[1]
            vt[o:o + n] = t[2]
        p.cw_gidx.append(_idx16(gt, Tc))
        p.cw_row.append(_row_tile(rt, Tc))
        p.cw_val.append(_val_tile(vt, Tc))

    # ---------------- final tokens (bucket x window grid-S) ----------------
    nodes = np.concatenate([row, col])
    cells = np.concatenate([np.arange(E), np.arange(E)])
    rk = nodes // RN
    per_rank_f = []
    for r in range(NCORES):
        sel = rk == r
        cc = cells[sel]
        cc = (cc // RE) * REP + cc % RE
        rl = nodes[sel] - r * RN
        b = cc // BUCK
        w = rl // GWIN
        d = {}
        for bb in range(NBUCK_E):
            for wi in range(NWIN_N):
                m = (b == bb) & (w == wi)
                d[(bb, wi)] = (cc[m] - bb * BUCK, rl[m], None)
        per_rank_f.append(d)
    fkeys = [(bb, wi) for bb in range(NBUCK_E) for wi in range(NWIN_N)]
    fcs, foffs, Tf = _prep_grid_s(per_rank_f, fkeys)
    p.f_keys = fkeys
    p.f_slots = fcs
    p.f_offs = foffs
    p.f_Tf = Tf
    p.f_idx = []
    p.f_row = []
    for r in range(NCORES):
        toks = np.zeros(Tf, np.int64)
        rows_all = np.full(Tf, -1, np.int64)
        for ki, key in enumerate(fkeys):
            idx, rows, _ = per_rank_f[r][key]
            o = foffs[ki]
            n = len(idx)
            toks[o:o + n] = idx
            rows_all[o:o + n] = rows - key[1] * GWIN
        p.f_idx.append(_idx16(toks, Tf))
        p.f_row.append(_row_tile(rows_all, Tf))

    # ---------------- pack everything into one blob per rank --------------
    # (the PJRT/axon path has ~1.6ms per-input-tensor overhead per execute;
    # bytes are nearly free -> ship ONE int16 tensor per rank)
    p.blob_layout = _blob_layout(Tg, Tc, Tf)
    tot = p.blob_layout["_total"]
    p.blob = []
    for r in range(NCORES):
        parts = {
            "x_sh": p.x_sh[r], "xe_sh": p.xe_sh[r],
            "gnnW": p.gnnW, "W0": p.W0, "W1": p.W1, "W2": p.W2,
            "ident": p.ident, "iota": p.iota, "iota256": p.iota256,
            "gnn_idx": p.gnn_idx[r], "gnn_row": p.gnn_row[r],
            "cw_gidx": p.cw_gidx[r], "cw_row": p.cw_row[r],
            "cw_val": p.cw_val[r],
            "f_idx": p.f_idx[r], "f_row": p.f_row[r],
        }
        b = np.empty(tot, np.int16)
        for name, spec in p.blob_layout.items():
            if name == "_total":
                continue
            off, shape, _ = spec
            a = parts[name]
            n = a.size
            b[off:off + n] = np.ascontiguousarray(a).view(np.int16).ravel()
        p.blob.append(b)

    # structure key for compile cache
    p.skey = (Tg, tuple(cs), Tc, tuple(cw_slots), Tf, tuple(fcs))
    return p


def _blob_layout(Tg, Tc, Tf):
    """name -> (element offset, shape, dtype_tag); all 2-byte elements."""
    specs = [
        ("x_sh", (RNP, D), "f16"),
        ("xe_sh", (REP, D), "f16"),
        ("gnnW", (D, L * D), "f16"),
        ("W0", (D, L * D), "f16"),
        ("W1", (D, L * D), "f16"),
        ("W2", (D, L * D), "f16"),
        ("ident", (D, D), "f16"),
        ("iota", (D, D), "f16"),
        ("iota256", (D, 2 * D), "f16"),
        ("gnn_idx", (16, Tg // 16), "i16"),
        ("gnn_row", (128, Tg // 128), "f16"),
        ("cw_gidx", (16, Tc // 16), "i16"),
        ("cw_row", (128, Tc // 128), "f16"),
        ("cw_val", (128, Tc // 128), "f16"),
        ("f_idx", (16, Tf // 16), "i16"),
        ("f_row", (128, Tf // 128), "f16"),
    ]
    layout = {}
    off = 0
    for name, shape, tag in specs:
        n = int(np.prod(shape))
        layout[name] = (off, shape, tag)
        off += n
    layout["_total"] = off
    return layout


# ---------------------------------------------------------------------------
# Device program
# ---------------------------------------------------------------------------

def _build(p, stage=6):
    # stage (cumulative, for HW-time bisection; 6 = full kernel):
    # 0 const loads+out, 1 +table AGs, 2 +CWNN L0 gather/scatter,
    # 3 +W phase L0 + he AG, 4 +GNN L0, 5 +all layers, 6 +final
    import concourse.bacc as bacc
    import concourse.tile as tile
    import concourse.mybir as mybir
    F16 = mybir.dt.float16
    F32 = mybir.dt.float32
    I16 = mybir.dt.int16
    RELU = mybir.ActivationFunctionType.Relu
    COPY = mybir.ActivationFunctionType.Copy
    EQ = mybir.AluOpType.is_equal

    nc = bacc.Bacc("TRN2", target_bir_lowering=False, debug=False,
                   num_devices=NCORES)

    # ---- I/O: ONE packed int16 blob per rank (see _blob_layout) ----
    layout = p.blob_layout
    blob = nc.dram_tensor("blob", [layout["_total"]], I16,
                          kind="ExternalInput")

    def bv(name):
        off, shape, tag = layout[name]
        n = int(np.prod(shape))
        ap = blob.ap()[off:off + n].rearrange("(a b) -> a b", b=shape[1])
        return ap if tag == "i16" else ap.bitcast(F16)

    x_sh_ap = bv("x_sh")
    xe_sh_ap = bv("xe_sh")
    out = nc.dram_tensor("out", [RNP, 2 * D], F16, kind="ExternalOutput")

    # ---- internal DRAM ----
    x16 = nc.dram_tensor("x16_d", [NP_TBL, D], F16, addr_space="Shared")
    xe16 = nc.dram_tensor("xe16_d", [EP_TBL, D], F16, addr_space="Shared")
    h_full = [None,
              nc.dram_tensor("h_full1", [NP_TBL, D], F16, addr_space="Shared"),
              nc.dram_tensor("h_full2", [NP_TBL, D], F16, addr_space="Shared")]
    he_full = [None,
               nc.dram_tensor("he_full1", [EP_TBL, D], F16, addr_space="Shared"),
               nc.dram_tensor("he_full2", [EP_TBL, D], F16, addr_space="Shared"),
               nc.dram_tensor("he_full3", [EP_TBL, D], F16, addr_space="Shared")]
    h_bounce = [nc.dram_tensor(f"h_bounce{i}", [RNP, D], F16) for i in range(2)]
    he_bounce = [nc.dram_tensor(f"he_bounce{i}", [REP, D], F16)
                 for i in range(3)]
    heT_d = [nc.dram_tensor(f"heT{i}", [D, REP], F16) for i in range(2)]
    x_bnc = nc.dram_tensor("x_bnc", [RNP, D], F16)
    xe_bnc = nc.dram_tensor("xe_bnc", [REP, D], F16)

    rg = [list(range(NCORES))]

    with tile.TileContext(nc) as tc:
        with tc.tile_pool(name="const", bufs=1) as cpool, \
             tc.tile_pool(name="gat", bufs=3) as gpool, \
             tc.tile_pool(name="sbld", bufs=2) as spool, \
             tc.tile_pool(name="hTp", bufs=2) as hpool, \
             tc.tile_pool(name="wph", bufs=3) as wpool, \
             tc.tile_pool(name="agg", bufs=2) as apool, \
             tc.tile_pool(name="tt", bufs=4) as tpool, \
             tc.tile_pool(name="ps_s", bufs=2, space="PSUM") as pps, \
             tc.tile_pool(name="ps_w", bufs=2, space="PSUM") as ppw:

            ident = cpool.tile([D, D], F16, tag="ident")
            nc.sync.dma_start(out=ident[:], in_=bv("ident"))
            iota = cpool.tile([D, D], F16, tag="iota")
            nc.sync.dma_start(out=iota[:], in_=bv("iota"))
            gW = cpool.tile([D, L * D], F16, tag="gW")
            nc.sync.dma_start(out=gW[:], in_=bv("gnnW"))
            w0 = cpool.tile([D, L * D], F16, tag="w0")
            nc.sync.dma_start(out=w0[:], in_=bv("W0"))
            w1 = cpool.tile([D, L * D], F16, tag="w1")
            nc.sync.dma_start(out=w1[:], in_=bv("W1"))
            w2 = cpool.tile([D, L * D], F16, tag="w2")
            nc.sync.dma_start(out=w2[:], in_=bv("W2"))

            # resident index tiles: expand [16, W] -> [128, W] (x8 stripes)
            def expand_idx(src_ap, W, tag):
                t = cpool.tile([128, W], I16, tag=tag)
                for j in range(8):
                    nc.sync.dma_start(out=t[16 * j:16 * (j + 1), :],
                                      in_=src_ap)
                return t

            iota256 = cpool.tile([D, 2 * D], F16, tag="iota256")
            nc.sync.dma_start(out=iota256[:], in_=bv("iota256"))
            gnn_it = expand_idx(bv("gnn_idx"), p.gnn_Tg // 16, "gnn_it")
            cw_git = expand_idx(bv("cw_gidx"), p.cw_Tc // 16, "cw_git")
            f_it = expand_idx(bv("f_idx"), p.f_Tf // 16, "f_it")
            cw_vt = cpool.tile([128, p.cw_Tc // 128], F16, tag="cw_vt")
            nc.sync.dma_start(out=cw_vt[:], in_=bv("cw_val"))
            cw_rt = cpool.tile([128, p.cw_Tc // 128], F16, tag="cw_rt")
            nc.sync.dma_start(out=cw_rt[:], in_=bv("cw_row"))
            gnn_rt = cpool.tile([128, p.gnn_Tg // 128], F16, tag="gnn_rt")
            nc.sync.dma_start(out=gnn_rt[:], in_=bv("gnn_row"))
            f_rt = cpool.tile([128, p.f_Tf // 128], F16, tag="f_rt")
            nc.sync.dma_start(out=f_rt[:], in_=bv("f_row"))

            # SBUF-resident transposed CWNN accumulators [D, REP]
            agg_do = cpool.tile([D, REP], F16, tag="agg_do")
            agg_up = cpool.tile([D, REP], F16, tag="agg_up")

            # transposed local x: hT [D, RNP]
            hT = hpool.tile([D, RNP], F16, tag="hT")
            for wv in range(RNP // WWIN):
                nc.scalar.dma_start_transpose(
                    hT[:, wv * WWIN:(wv + 1) * WWIN],
                    x_sh_ap[wv * WWIN:(wv + 1) * WWIN, :])

            # replicated tables via AllGather (collectives cannot read IO
            # tensors -> bounce through internal DRAM first)
            if stage >= 1:
                nc.sync.dma_start(out=x_bnc.ap(), in_=x_sh_ap)
                for ch in range(4):
                    r0 = ch * (REP // 4)
                    rn = (REP // 4) if ch < 3 else REP - 3 * (REP // 4)
                    nc.sync.dma_start(out=xe_bnc.ap()[r0:r0 + rn, :],
                                      in_=xe_sh_ap[r0:r0 + rn, :])
                nc.gpsimd.collective_compute(
                    "AllGather", mybir.AluOpType.bypass, replica_groups=rg,
                    ins=[x_bnc.ap().opt()], outs=[x16.ap().opt()])
                nc.gpsimd.collective_compute(
                    "AllGather", mybir.AluOpType.bypass, replica_groups=rg,
                    ins=[xe_bnc.ap().opt()], outs=[xe16.ap().opt()])

            # ============ helper: grid-S aggregation ============
            def grid_s_agg(agg, n_win, offs, idx_tile, row_tile, tables,
                           phase):
                """agg: SBUF tile [128, n_win*128]; tables: list of
                (dram_tensor, row_lo, row_n, tok_lo, tok_hi) gather sources."""
                for (src, row_lo, row_n, t0, t1) in tables:
                    t = t0
                    while t < t1:
                        n = min(MAXTOK, t1 - t)
                        G = n // 128
                        g = gpool.tile([128, MAXTOK // 128, D], F16,
                                       tag="g")
                        nc.gpsimd.dma_gather(
                            g[:, :G, :], src.ap()[row_lo:row_lo + row_n, :],
                            idx_tile[:, t // 16:(t + n) // 16], n, n, D)
                        # one batched S build for all G chunks of this gather
                        stb = spool.tile([128, MAXTOK // 128, GWIN], F16,
                                         tag="s1h")
                        nc.vector.tensor_tensor(
                            out=stb[:, :G, :],
                            in0=iota[:].unsqueeze(1)
                                .broadcast_to([128, G, GWIN]),
                            in1=row_tile[:, t // 128:(t + n) // 128]
                                .unsqueeze(2).broadcast_to([128, G, GWIN]),
                            op=EQ)
                        for c in range(G):
                            tok = t + c * 128
                            ki = int(np.searchsorted(offs, tok, side="right")) - 1
                            wi = ki % n_win if phase == "f" else ki
                            ps = pps.tile([128, GWIN], F32, tag="ps_s")
                            nc.tensor.matmul(
                                ps[:], lhsT=g[:, c, :], rhs=stb[:, c, :],
                                start=True, stop=True)
                            nc.vector.tensor_tensor(
                                out=agg[:, wi * GWIN:(wi + 1) * GWIN],
                                in0=agg[:, wi * GWIN:(wi + 1) * GWIN],
                                in1=ps[:], op=mybir.AluOpType.add)
                        t += n

            # ============ interleaved layers ============
            n_layers = L if stage >= 5 else (1 if stage >= 2 else 0)
            CWWIN = p.CWWIN
            for i in range(n_layers):
                # ---------- CWNN layer i ----------
                he_table = xe16 if i == 0 else he_full[i]
                heT_nxt = heT_d[i % 2]

                nc.vector.memset(agg_do[:], 0.0)
                nc.vector.memset(agg_up[:], 0.0)

                # gather ranges are contiguous per (lap, bucket); chunks map
                # statically to (lap, b, window) cells via cw_offs
                coffs = p.cw_offs
                lb_ranges = []
                for lap in range(2):
                    for bb in range(NBUCK_E):
                        k0 = (lap * NBUCK_E + bb) * (REP // CWWIN)
                        k1 = k0 + (REP // CWWIN)
                        t0, t1 = int(coffs[k0]), int(coffs[k1])
                        if t1 > t0:
                            lb_ranges.append((bb, t0, t1))
                for (bb, t0, t1) in lb_ranges:
                    row_lo = bb * BUCK
                    row_n = min(BUCK, EP_TBL - row_lo)
                    t = t0
                    while t < t1:
                        n = min(MAXTOK, t1 - t)
                        G = n // 128
                        g = gpool.tile([128, MAXTOK // 128, D], F16, tag="g")
                        nc.gpsimd.dma_gather(
                            g[:, :G, :], he_table.ap()[row_lo:row_lo + row_n, :],
                            cw_git[:, t // 16:(t + n) // 16], n, n, D)
                        nc.vector.tensor_tensor(
                            out=g[:, :G, :], in0=g[:, :G, :],
                            in1=cw_vt[:, t // 128:(t + n) // 128]
                                .unsqueeze(2).broadcast_to([128, G, D]),
                            op=mybir.AluOpType.mult)
                        # one batched S build for all G chunks of this gather
                        stb = spool.tile([128, MAXTOK // 128, CWWIN], F16,
                                         tag="s2h")
                        nc.vector.tensor_tensor(
                            out=stb[:, :G, :],
                            in0=iota256[:].unsqueeze(1)
                                .broadcast_to([128, G, CWWIN]),
                            in1=cw_rt[:, t // 128:(t + n) // 128]
                                .unsqueeze(2).broadcast_to([128, G, CWWIN]),
                            op=EQ)
                        for c in range(G):
                            tok = t + c * 128
                            ki = int(np.searchsorted(coffs, tok,
                                                     side="right")) - 1
                            lap, _, ww = p.cw_keys[ki]
                            agg = agg_do if lap == 0 else agg_up
                            ps = pps.tile([128, CWWIN], F32, tag="ps_c")
                            nc.tensor.matmul(ps[:], lhsT=g[:, c, :],
                                             rhs=stb[:, c, :],
                                             start=True, stop=True)
                            nc.vector.tensor_tensor(
                                out=agg[:, ww * CWWIN:(ww + 1) * CWWIN],
                                in0=agg[:, ww * CWWIN:(ww + 1) * CWWIN],
                                in1=ps[:], op=mybir.AluOpType.add)
                        t += n

                if stage < 3:
                    break
                # W phase: he_next = relu(he@W0 + acc_do@W1 + acc_up@W2)
                # (agg_do/agg_up already live in SBUF, transposed)
                for w in range(REP // WWIN):
                    c0 = w * WWIN
                    cn = WWIN
                    hw = wpool.tile([128, WWIN], F16, tag="hw")
                    if i == 0:
                        nc.scalar.dma_start_transpose(
                            hw[:, :cn], xe_sh_ap[c0:c0 + cn, :])
                    else:
                        nc.sync.dma_start(
                            out=hw[:, :cn],
                            in_=heT_d[(i + 1) % 2].ap()[:, c0:c0 + cn])
                    ps = ppw.tile([128, WWIN], F32, tag="ps_w")
                    nc.tensor.matmul(ps[:, :cn], lhsT=w0[:, i * D:(i + 1) * D],
                                     rhs=hw[:, :cn], start=True, stop=False)
                    nc.tensor.matmul(ps[:, :cn], lhsT=w1[:, i * D:(i + 1) * D],
                                     rhs=agg_do[:, c0:c0 + cn],
                                     start=False, stop=False)
                    nc.tensor.matmul(ps[:, :cn], lhsT=w2[:, i * D:(i + 1) * D],
                                     rhs=agg_up[:, c0:c0 + cn],
                                     start=False, stop=True)
                    hn = wpool.tile([128, WWIN], F16, tag="hn")
                    nc.scalar.activation(hn[:, :cn], ps[:, :cn], RELU)
                    nc.sync.dma_start(out=heT_nxt.ap()[:, c0:c0 + cn],
                                      in_=hn[:, :cn])
                    # row-major blocks for AllGather input via PE transpose
                    rows = tpool.tile([128, WWIN // 128, D], F16, tag="cw_rr")
                    for tt_i in range(WWIN // 128):
                        r0 = tt_i * 128
                        pst = pps.tile([128, D], F32, tag="ps_t")
                        nc.tensor.matmul(pst[:], lhsT=hn[:, r0:r0 + 128],
                                         rhs=ident[:], start=True, stop=True)
                        nc.scalar.activation(rows[:, tt_i, :], pst[:], COPY)
                    nc.sync.dma_start(
                        out=he_bounce[i].ap()[c0:c0 + cn, :]
                            .rearrange("(g q) d -> q g d", q=128),
                        in_=rows[:])

                nc.gpsimd.collective_compute(
                    "AllGather", mybir.AluOpType.bypass, replica_groups=rg,
                    ins=[he_bounce[i].ap().opt()],
                    outs=[he_full[i + 1].ap().opt()])

                if stage < 4:
                    break
                # ---------- GNN layer i ----------
                h_table = x16 if i == 0 else h_full[i]
                agg = apool.tile([128, RNP], F16, tag="gagg")
                nc.vector.memset(agg[:], 0.0)
                grid_s_agg(agg, NWIN_N, p.gnn_win_offs_np, gnn_it, gnn_rt,
                           [(h_table, 0, NP_TBL, 0, p.gnn_Tg)], "g")
                nc.vector.tensor_tensor(out=agg[:], in0=agg[:],
                                        in1=hT[:],
                                        op=mybir.AluOpType.add)
                hT = hpool.tile([D, RNP], F16, tag="hT")
                for w in range(_ceil(RNP, WWIN)):
                    c0 = w * WWIN
                    cn = min(WWIN, RNP - c0)
                    ps = ppw.tile([128, WWIN], F32, tag="ps_w")
                    nc.tensor.matmul(ps[:, :cn], lhsT=gW[:, i * D:(i + 1) * D],
                                     rhs=agg[:, c0:c0 + cn],
                                     start=True, stop=True)
                    nc.scalar.activation(hT[:, c0:c0 + cn], ps[:, :cn], RELU)
                if i < L - 1:
                    for t in range(RNP // 128):
                        r0 = t * 128
                        tt = tpool.tile([128, 128], F16, tag="g_tt")
                        nc.scalar.dma_start_transpose(
                            tt[:], hT[:, r0:r0 + 128])
                        nc.sync.dma_start(
                            out=h_bounce[i].ap()[r0:r0 + 128, :],
                            in_=tt[:])
                    nc.gpsimd.collective_compute(
                        "AllGather", mybir.AluOpType.bypass, replica_groups=rg,
                        ins=[h_bounce[i].ap().opt()],
                        outs=[h_full[i + 1].ap().opt()])

            # ============ final: xed = segsum(he3, row) + segsum(he3, col) ==
            fagg = apool.tile([128, RNP], F16, tag="fagg")
            nc.vector.memset(fagg[:], 0.0)
            ftables = []
            for bi, bb in enumerate(range(NBUCK_E) if stage >= 6 else []):
                klo = bi * NWIN_N
                t0 = int(p.f_offs[klo])
                t1 = int(p.f_offs[klo + NWIN_N])
                row_lo = bb * BUCK
                row_n = min(BUCK, EP_TBL - row_lo)
                if t1 > t0:
                    ftables.append((he_full[3], row_lo, row_n, t0, t1))
            grid_s_agg(fagg, NWIN_N, p.f_offs, f_it, f_rt, ftables, "f")

            # output: [RNP, 0:128] = h3 rows, [RNP, 128:256] = xed
            for t in range(NWIN_N):
                c0 = t * 128
                psx = pps.tile([128, 128], F32, tag="ps_s")
                nc.tensor.matmul(psx[:], lhsT=hT[:, c0:c0 + 128], rhs=ident[:],
                                 start=True, stop=True)
                ox = tpool.tile([128, 128], F16, tag="tt16")
                nc.vector.tensor_copy(ox[:], psx[:])
                nc.sync.dma_start(out=out.ap()[c0:c0 + 128, 0:D], in_=ox[:])
                psy = pps.tile([128, 128], F32, tag="ps_s")
                nc.tensor.matmul(psy[:], lhsT=fagg[:, c0:c0 + 128],
                                 rhs=ident[:], start=True, stop=True)
                oy = tpool.tile([128, 128], F16, tag="tt16")
                nc.vector.tensor_copy(oy[:], psy[:])
                nc.sync.dma_start(out=out.ap()[c0:c0 + 128, D:2 * D], in_=oy[:])

    nc.compile()
    return nc


# ---------------------------------------------------------------------------
# PJRT runner (axon path; no /dev/neuron* on client)
# ---------------------------------------------------------------------------

def _make_runner(nc):
    import jax
    import time
    from jax.sharding import Mesh, PartitionSpec
    from jax.experimental.shard_map import shard_map
    import concourse.mybir as mybir
    import concourse.bass2jax as bass2jax
    from concourse.bass2jax import _bass_exec_p, install_neuronx_cc_hook

    install_neuronx_cc_hook()
    partition_name = nc.partition_id_tensor.name if nc.partition_id_tensor else None

    in_names, out_names, out_avals, zero_outs = [], [], [], []
    for alloc in nc.m.functions[0].allocations:
        if not isinstance(alloc, mybir.MemoryLocationSet):
            continue
        name = alloc.memorylocations[0].name
        if alloc.kind == "ExternalInput":
            if name != partition_name:
                in_names.append(name)
        elif alloc.kind == "ExternalOutput":
            out_names.append(name)
            shape = tuple(alloc.tensor_shape)
            dtype = mybir.dt.np(alloc.dtype)
            out_avals.append(jax.core.ShapedArray(shape, dtype))
            zero_outs.append(np.zeros(shape, dtype))
    n_params = len(in_names)
    all_in_names = list(in_names) + list(out_names)
    if partition_name is not None:
        all_in_names.append(partition_name)

    def _body(*args):
        operands = list(args)
        if partition_name is not None:
            operands.append(bass2jax.partition_id_tensor())
        outs = _bass_exec_p.bind(
            *operands,
            out_avals=tuple(out_avals),
            in_names=tuple(all_in_names),
            out_names=tuple(out_names),
            lowering_input_output_aliases=(),
            sim_require_finite=True,
            sim_require_nnan=True,
            nc=nc,
        )
        return tuple(outs)

    devices = jax.devices()[:NCORES]
    mesh = Mesh(np.asarray(devices), ("core",))
    in_specs = (PartitionSpec("core"),) * (n_params + len(out_names))
    out_specs = (PartitionSpec("core"),) * len(out_names)
    sharded = jax.jit(
        shard_map(_body, mesh=mesh, in_specs=in_specs, out_specs=out_specs,
                  check_rep=False),
        keep_unused=True,
    )

    def run_fn(in_maps, iters=1):
        per_core = [[np.asarray(m[name]) for name in in_names] for m in in_maps]
        concat_in = [np.concatenate([per_core[c][i] for c in range(NCORES)], axis=0)
                     for i in range(n_params)]
        concat_zeros = [np.zeros((NCORES * z.shape[0], *z.shape[1:]), z.dtype)
                        for z in zero_outs]
        dev_in = [jax.device_put(a) for a in concat_in]
        dev_zero = [jax.device_put(z) for z in concat_zeros]
        out = sharded(*dev_in, *dev_zero)
        jax.block_until_ready(out)
        t0 = time.perf_counter()
        if iters > 1:
            for _ in range(iters):
                out = sharded(*dev_in, *dev_zero)
            jax.block_until_ready(out)
            dt = (time.perf_counter() - t0) / iters
        else:
            dt = 0.0
        results = [
            {name: np.asarray(out[i]).reshape(NCORES, *out_avals[i].shape)[c]
             for i, name in enumerate(out_names)}
            for c in range(NCORES)
        ]
        return results, dt

    return run_fn

# ---------------------------------------------------------------------------

_CACHE = {}


def _get_runner(p):
    key = p.skey
    if key in _CACHE:
        return _CACHE[key]
    # np arrays needed by builder
    offs = np.zeros(NWIN_N + 1, np.int64)
    offs[1:] = np.cumsum(p.gnn_win_slots)
    p.gnn_win_offs_np = offs
    nc = _build(p)
    run_fn = _make_runner(nc)
    _CACHE[key] = run_fn
    return run_fn


def kernel(**inputs):
    p = _preprocess(**inputs)
    run_fn = _get_runner(p)
    in_maps = [{"blob": p.blob[r]} for r in range(NCORES)]
    results, dt = run_fn(in_maps, iters=1)
    kernel.last_dt = dt
    kernel.run_fn = run_fn
    kernel.in_maps = in_maps
    outs = [results[r]["out"][:RN] for r in range(NCORES)]
    return np.concatenate(outs, axis=0).astype(np.float32)


# revision 31
# speedup vs baseline: 1.1378x; 1.1378x over previous
"""Trainium2 Bass kernel for nn_CellNetwork (GNN + CWNN message passing).

Self-contained: takes FULL inputs, shards across 8 NeuronCores internally,
returns FULL output [20000, 256] fp32.

Strategy (SPMD, one program for all 8 ranks; per-rank data differs but all
instruction shapes are rank-uniform via max-over-ranks padding):
  - fp16 on device, fp32 PSUM accumulation.
  - Host->device traffic is minimized: only per-rank feature shards and
    compact int16 token indices ship. Replicated gather tables are built on
    device via AllGather; one-hot S matrices are built on the fly from 2-byte
    row indices (iota compare on DVE); accumulators are zeroed on device.
  - GNN over nodes: sharded by node (2500/rank). Segment-sum via "S-matrix"
    PE matmuls on a fixed grid of 128-row windows; h tables re-replicated
    per layer via AllGather.
  - CWNN over cells: sharded by cell (25000/rank). Messages gathered from a
    replicated he table (dma_gather, int16 indices -> 7 x 32768-row buckets),
    scaled by val (DVE broadcast mult), scatter-added (dma_scatter_add) into
    local accumulators; duplicate-row races avoided by splitting tokens into
    "rounds" with unique rows, serialized by Tile's WAW tracking.
  - Final dual scatter-add of cells to nodes: grid-S matmuls over he3.
"""
import sys
import numpy as np

sys.path.insert(0, "/opt/trn_rl_repo")

N = 20000
E = 200000
D = 128
NNZ = 400000
L = 3
NCORES = 8
RN = N // NCORES          # 2500 nodes per rank
RE = E // NCORES          # 25000 cells per rank
BUCK = 32768              # gather index window (int16 limit)
NBUCK_E = (NCORES * 25088 + BUCK - 1) // BUCK   # 7 (on padded table)
GWIN = 128                # grid-S window (rows per PSUM tile)
NWIN_N = (RN + GWIN - 1) // GWIN   # 20 windows for 2500 local nodes
RNP = NWIN_N * GWIN       # 2560 padded local nodes
WWIN = 512                # W-phase window (cols per matmul)
MAXTOK = 1024             # max tokens per dma_gather/scatter (SWDGE ring = 1024 descs)
REP = 25088               # cells per rank, padded to 49*512
EP_TBL = NCORES * REP     # 200704-row padded cell table
NP_TBL = NCORES * RNP     # 20480-row padded node table


def _pad128(n):
    return (n + 127) & ~127


def _ceil(a, b):
    return (a + b - 1) // b


# ---------------------------------------------------------------------------
# Host-side preprocessing
# ---------------------------------------------------------------------------

def _idx16(tokens, n_slots):
    """[16, n_slots/16] int16: token t at (t%16, t//16). Expanded to 128
    partitions (x8 replication) on device."""
    W = n_slots // 16
    arr = np.zeros((16, W), np.int16)
    t = np.arange(len(tokens))
    arr[t % 16, t // 16] = tokens.astype(np.int16)
    return arr


def _val_tile(vals, n_slots):
    """[128, n_slots/128] fp16: token t at (t%128, t//128)."""
    G = n_slots // 128
    arr = np.zeros((128, G), np.float16)
    t = np.arange(len(vals))
    arr[t % 128, t // 128] = vals.astype(np.float16)
    return arr


def _row_tile(rows, n_slots):
    """[128, n_slots/128] fp16: window-row of token t at (t%128, t//128);
    padded slots get -1 (matches no iota value -> zero S column)."""
    G = n_slots // 128
    arr = np.full((128, G), -1.0, np.float16)
    t = np.arange(len(rows))
    arr[t % 128, t // 128] = rows.astype(np.float16)
    return arr


def _prep_grid_s(per_rank_tokens, n_cells_keys):
    """Grid-S packing: tokens grouped into cells (ordered by key tuple list
    n_cells_keys); each cell padded to max-over-ranks chunk count."""
    nr = len(per_rank_tokens)
    cell_slots = []
    for key in n_cells_keys:
        mx = 0
        for r in range(nr):
            t = per_rank_tokens[r].get(key)
            if t is not None:
                mx = max(mx, len(t[0]))
        cell_slots.append(_pad128(mx))
    total = int(np.sum(cell_slots))
    offs = np.zeros(len(cell_slots) + 1, np.int64)
    offs[1:] = np.cumsum(cell_slots)
    return cell_slots, offs, total


class _Prep:
    pass


def _preprocess(x, xe, gnn_W, cwnn_W0, cwnn_W1, cwnn_W2, Ldo_val, Lup_val,
                edges, row, col, Ldo_idx, Lup_idx):
    p = _Prep()
    f16 = np.float16
    x16 = np.asarray(x, np.float32).astype(f16)
    xe16 = np.asarray(xe, np.float32).astype(f16)
    # per-rank padded shards
    p.x_sh = []
    p.xe_sh = []
    for r in range(NCORES):
        xs = np.zeros((RNP, D), f16)
        xs[:RN] = x16[r * RN:(r + 1) * RN]
        p.x_sh.append(xs)
        xes = np.zeros((REP, D), f16)
        xes[:RE] = xe16[r * RE:(r + 1) * RE]
        p.xe_sh.append(xes)
    p.gnnW = np.ascontiguousarray(
        np.asarray(gnn_W, np.float32).astype(f16).transpose(1, 0, 2).reshape(D, L * D))
    p.W0 = np.ascontiguousarray(
        np.asarray(cwnn_W0, np.float32).astype(f16).transpose(1, 0, 2).reshape(D, L * D))
    p.W1 = np.ascontiguousarray(
        np.asarray(cwnn_W1, np.float32).astype(f16).transpose(1, 0, 2).reshape(D, L * D))
    p.W2 = np.ascontiguousarray(
        np.asarray(cwnn_W2, np.float32).astype(f16).transpose(1, 0, 2).reshape(D, L * D))
    p.ident = np.eye(D, dtype=f16)
    p.iota = np.tile(np.arange(128, dtype=f16)[None, :], (128, 1))
    p.iota256 = np.tile(np.arange(256, dtype=f16)[None, :], (128, 1))

    edges = np.asarray(edges, np.int64)
    row = np.asarray(row, np.int64)
    col = np.asarray(col, np.int64)
    Ldo_idx = np.asarray(Ldo_idx, np.int64)
    Lup_idx = np.asarray(Lup_idx, np.int64)

    # ---------------- GNN tokens (grid-S, 1 bucket, 20 windows) -----------
    src, dst = edges[0], edges[1]
    rk = dst // RN
    per_rank = []
    for r in range(NCORES):
        sel = rk == r
        s_src = src[sel]
        s_src = (s_src // RN) * RNP + s_src % RN
        s_row = dst[sel] - r * RN
        w = s_row // GWIN
        order = np.argsort(w, kind="stable")
        s_src, s_row, w = s_src[order], s_row[order], w[order]
        d = {}
        for wi in range(NWIN_N):
            m = w == wi
            d[wi] = (s_src[m], s_row[m], None)
        per_rank.append(d)
    keys = list(range(NWIN_N))
    cs, offs, Tg = _prep_grid_s(per_rank, keys)
    p.gnn_win_slots = cs          # slots per window (multiple of 128)
    p.gnn_Tg = Tg
    p.gnn_idx = []
    p.gnn_row = []
    for r in range(NCORES):
        toks = np.zeros(Tg, np.int64)
        rows_all = np.full(Tg, -1, np.int64)
        for wi in range(NWIN_N):
            idx, rows, _ = per_rank[r][wi]
            o = offs[wi]
            n = len(idx)
            toks[o:o + n] = idx
            rows_all[o:o + n] = rows - wi * GWIN
        p.gnn_idx.append(_idx16(toks, Tg))
        p.gnn_row.append(_row_tile(rows_all, Tg))

    # ---------------- CWNN tokens (lap x bucket x 256-row window grid) -----
    # token: (col_global, row_local, val) ; lap 0 = Ldo, 1 = Lup
    # Receive side is S-matmul into SBUF agg (no dma_scatter_add): tokens
    # sorted by (lap, bucket, window); each cell padded to max-over-ranks
    # (multiple of 128); duplicates within a chunk are fine (PE sums them).
    lap_data = [(Ldo_idx, np.asarray(Ldo_val)), (Lup_idx, np.asarray(Lup_val))]
    CWWIN = 256
    NWIN_E = REP // CWWIN     # 98
    cell_tokens = [dict() for _ in range(NCORES)]  # key (lap, b, w)
    for lap, (lidx, lval) in enumerate(lap_data):
        lrow, lcol = lidx[0], lidx[1]
        rr = lrow // RE
        for r in range(NCORES):
            sel = rr == r
            c_g = lcol[sel]
            c_g = (c_g // RE) * REP + c_g % RE
            r_l = lrow[sel] - r * RE
            v = lval[sel]
            b = c_g // BUCK
            w = r_l // CWWIN
            key = b * NWIN_E + w
            order = np.argsort(key, kind="stable")
            c_g, r_l, v, key = c_g[order], r_l[order], v[order], key[order]
            for bb in range(NBUCK_E):
                for ww in range(NWIN_E):
                    m = key == bb * NWIN_E + ww
                    if m.any():
                        cell_tokens[r][(lap, bb, ww)] = (
                            c_g[m] - bb * BUCK, r_l[m] - ww * CWWIN, v[m])
    cw_keys = [(lap, bb, ww) for lap in range(2) for bb in range(NBUCK_E)
               for ww in range(NWIN_E)]
    cw_slots = []
    for key in cw_keys:
        mx = 0
        for r in range(NCORES):
            t = cell_tokens[r].get(key)
            if t is not None:
                mx = max(mx, len(t[0]))
        cw_slots.append(_pad128(mx))
    Tc = int(np.sum(cw_slots))
    coffs = np.zeros(len(cw_slots) + 1, np.int64)
    coffs[1:] = np.cumsum(cw_slots)
    p.cw_keys = cw_keys
    p.cw_slots = cw_slots
    p.cw_offs = coffs
    p.cw_Tc = Tc
    p.CWWIN = CWWIN
    p.cw_gidx = []
    p.cw_row = []
    p.cw_val = []
    for r in range(NCORES):
        gt = np.zeros(Tc, np.int64)
        rt = np.full(Tc, -1, np.int64)
        vt = np.zeros(Tc, np.float32)
        for si, key in enumerate(cw_keys):
            t = cell_tokens[r].get(key)
            if t is None:
                continue
            o = coffs[si]
            n = len(t[0])
            gt[o:o + n] = t[0]
            rt[o:o + n] = t[1]
            vt[o:o + n] = t[2]
        p.cw_gidx.append(_idx16(gt, Tc))
        p.cw_row.append(_row_tile(rt, Tc))
        p.cw_val.append(_val_tile(vt, Tc))

    # ---------------- final tokens (bucket x window grid-S) ----------------
    nodes = np.concatenate([row, col])
    cells = np.concatenate([np.arange(E), np.arange(E)])
    rk = nodes // RN
    per_rank_f = []
    for r in range(NCORES):
        sel = rk == r
        cc = cells[sel]
        cc = (cc // RE) * REP + cc % RE
        rl = nodes[sel] - r * RN
        b = cc // BUCK
        w = rl // GWIN
        d = {}
        for bb in range(NBUCK_E):
            for wi in range(NWIN_N):
                m = (b == bb) & (w == wi)
                d[(bb, wi)] = (cc[m] - bb * BUCK, rl[m], None)
        per_rank_f.append(d)
    fkeys = [(bb, wi) for bb in range(NBUCK_E) for wi in range(NWIN_N)]
    fcs, foffs, Tf = _prep_grid_s(per_rank_f, fkeys)
    p.f_keys = fkeys
    p.f_slots = fcs
    p.f_offs = foffs
    p.f_Tf = Tf
    p.f_idx = []
    p.f_row = []
    for r in range(NCORES):
        toks = np.zeros(Tf, np.int64)
        rows_all = np.full(Tf, -1, np.int64)
        for ki, key in enumerate(fkeys):
            idx, rows, _ = per_rank_f[r][key]
            o = foffs[ki]
            n = len(idx)
            toks[o:o + n] = idx
            rows_all[o:o + n] = rows - key[1] * GWIN
        p.f_idx.append(_idx16(toks, Tf))
        p.f_row.append(_row_tile(rows_all, Tf))

    # ---------------- pack everything into one blob per rank --------------
    # (the PJRT/axon path has ~1.6ms per-input-tensor overhead per execute;
    # bytes are nearly free -> ship ONE int16 tensor per rank)
    p.blob_layout = _blob_layout(Tg, Tc, Tf)
    tot = p.blob_layout["_total"]
    p.blob = []
    for r in range(NCORES):
        parts = {
            "x_sh": p.x_sh[r], "xe_sh": p.xe_sh[r],
            "gnnW": p.gnnW, "W0": p.W0, "W1": p.W1, "W2": p.W2,
            "ident": p.ident, "iota": p.iota, "iota256": p.iota256,
            "gnn_idx": p.gnn_idx[r], "gnn_row": p.gnn_row[r],
            "cw_gidx": p.cw_gidx[r], "cw_row": p.cw_row[r],
            "cw_val": p.cw_val[r],
            "f_idx": p.f_idx[r], "f_row": p.f_row[r],
        }
        b = np.empty(tot, np.int16)
        for name, spec in p.blob_layout.items():
            if name == "_total":
                continue
            off, shape, _ = spec
            a = parts[name]
            n = a.size
            b[off:off + n] = np.ascontiguousarray(a).view(np.int16).ravel()
        p.blob.append(b)

    # structure key for compile cache
    p.skey = (Tg, tuple(cs), Tc, tuple(cw_slots), Tf, tuple(fcs))
    return p


def _blob_layout(Tg, Tc, Tf):
    """name -> (element offset, shape, dtype_tag); all 2-byte elements."""
    specs = [
        ("x_sh", (RNP, D), "f16"),
        ("xe_sh", (REP, D), "f16"),
        ("gnnW", (D, L * D), "f16"),
        ("W0", (D, L * D), "f16"),
        ("W1", (D, L * D), "f16"),
        ("W2", (D, L * D), "f16"),
        ("ident", (D, D), "f16"),
        ("iota", (D, D), "f16"),
        ("iota256", (D, 2 * D), "f16"),
        ("gnn_idx", (16, Tg // 16), "i16"),
        ("gnn_row", (128, Tg // 128), "f16"),
        ("cw_gidx", (16, Tc // 16), "i16"),
        ("cw_row", (128, Tc // 128), "f16"),
        ("cw_val", (128, Tc // 128), "f16"),
        ("f_idx", (16, Tf // 16), "i16"),
        ("f_row", (128, Tf // 128), "f16"),
    ]
    layout = {}
    off = 0
    for name, shape, tag in specs:
        n = int(np.prod(shape))
        layout[name] = (off, shape, tag)
        off += n
    layout["_total"] = off
    return layout


# ---------------------------------------------------------------------------
# Device program
# ---------------------------------------------------------------------------

def _build(p, stage=6):
    # stage (cumulative, for HW-time bisection; 6 = full kernel):
    # 0 const loads+out, 1 +table AGs, 2 +CWNN L0 gather/scatter,
    # 3 +W phase L0 + he AG, 4 +GNN L0, 5 +all layers, 6 +final
    import concourse.bacc as bacc
    import concourse.tile as tile
    import concourse.mybir as mybir
    F16 = mybir.dt.float16
    F32 = mybir.dt.float32
    I16 = mybir.dt.int16
    RELU = mybir.ActivationFunctionType.Relu
    COPY = mybir.ActivationFunctionType.Copy
    EQ = mybir.AluOpType.is_equal

    nc = bacc.Bacc("TRN2", target_bir_lowering=False, debug=False,
                   num_devices=NCORES)

    # ---- I/O: ONE packed int16 blob per rank (see _blob_layout) ----
    layout = p.blob_layout
    blob = nc.dram_tensor("blob", [layout["_total"]], I16,
                          kind="ExternalInput")

    def bv(name):
        off, shape, tag = layout[name]
        n = int(np.prod(shape))
        ap = blob.ap()[off:off + n].rearrange("(a b) -> a b", b=shape[1])
        return ap if tag == "i16" else ap.bitcast(F16)

    x_sh_ap = bv("x_sh")
    xe_sh_ap = bv("xe_sh")
    out = nc.dram_tensor("out", [RNP, 2 * D], F16, kind="ExternalOutput")

    # ---- internal DRAM ----
    x16 = nc.dram_tensor("x16_d", [NP_TBL, D], F16, addr_space="Shared")
    xe16 = nc.dram_tensor("xe16_d", [EP_TBL, D], F16, addr_space="Shared")
    h_full = [None,
              nc.dram_tensor("h_full1", [NP_TBL, D], F16, addr_space="Shared"),
              nc.dram_tensor("h_full2", [NP_TBL, D], F16, addr_space="Shared")]
    he_full = [None,
               nc.dram_tensor("he_full1", [EP_TBL, D], F16, addr_space="Shared"),
               nc.dram_tensor("he_full2", [EP_TBL, D], F16, addr_space="Shared"),
               nc.dram_tensor("he_full3", [EP_TBL, D], F16, addr_space="Shared")]
    h_bounce = [nc.dram_tensor(f"h_bounce{i}", [RNP, D], F16) for i in range(2)]
    he_bounce = [nc.dram_tensor(f"he_bounce{i}", [REP, D], F16)
                 for i in range(3)]
    heT_d = [nc.dram_tensor(f"heT{i}", [D, REP], F16) for i in range(2)]
    x_bnc = nc.dram_tensor("x_bnc", [RNP, D], F16)
    xe_bnc = nc.dram_tensor("xe_bnc", [REP, D], F16)

    rg = [list(range(NCORES))]

    with tile.TileContext(nc) as tc:
        with tc.tile_pool(name="const", bufs=1) as cpool, \
             tc.tile_pool(name="gat", bufs=3) as gpool, \
             tc.tile_pool(name="sbld", bufs=2) as spool, \
             tc.tile_pool(name="hTp", bufs=2) as hpool, \
             tc.tile_pool(name="wph", bufs=3) as wpool, \
             tc.tile_pool(name="agg", bufs=2) as apool, \
             tc.tile_pool(name="tt", bufs=4) as tpool, \
             tc.tile_pool(name="ps_s", bufs=2, space="PSUM") as pps, \
             tc.tile_pool(name="ps_c", bufs=4, space="PSUM") as ppc, \
             tc.tile_pool(name="ps_w", bufs=2, space="PSUM") as ppw:

            ident = cpool.tile([D, D], F16, tag="ident")
            nc.sync.dma_start(out=ident[:], in_=bv("ident"))
            iota = cpool.tile([D, D], F16, tag="iota")
            nc.sync.dma_start(out=iota[:], in_=bv("iota"))
            gW = cpool.tile([D, L * D], F16, tag="gW")
            nc.sync.dma_start(out=gW[:], in_=bv("gnnW"))
            w0 = cpool.tile([D, L * D], F16, tag="w0")
            nc.sync.dma_start(out=w0[:], in_=bv("W0"))
            w1 = cpool.tile([D, L * D], F16, tag="w1")
            nc.sync.dma_start(out=w1[:], in_=bv("W1"))
            w2 = cpool.tile([D, L * D], F16, tag="w2")
            nc.sync.dma_start(out=w2[:], in_=bv("W2"))

            # resident index tiles: expand [16, W] -> [128, W] (x8 stripes)
            def expand_idx(src_ap, W, tag):
                t = cpool.tile([128, W], I16, tag=tag)
                for j in range(8):
                    nc.sync.dma_start(out=t[16 * j:16 * (j + 1), :],
                                      in_=src_ap)
                return t

            iota256 = cpool.tile([D, 2 * D], F16, tag="iota256")
            nc.sync.dma_start(out=iota256[:], in_=bv("iota256"))
            gnn_it = expand_idx(bv("gnn_idx"), p.gnn_Tg // 16, "gnn_it")
            cw_git = expand_idx(bv("cw_gidx"), p.cw_Tc // 16, "cw_git")
            f_it = expand_idx(bv("f_idx"), p.f_Tf // 16, "f_it")
            cw_vt = cpool.tile([128, p.cw_Tc // 128], F16, tag="cw_vt")
            nc.sync.dma_start(out=cw_vt[:], in_=bv("cw_val"))
            cw_rt = cpool.tile([128, p.cw_Tc // 128], F16, tag="cw_rt")
            nc.sync.dma_start(out=cw_rt[:], in_=bv("cw_row"))
            gnn_rt = cpool.tile([128, p.gnn_Tg // 128], F16, tag="gnn_rt")
            nc.sync.dma_start(out=gnn_rt[:], in_=bv("gnn_row"))
            f_rt = cpool.tile([128, p.f_Tf // 128], F16, tag="f_rt")
            nc.sync.dma_start(out=f_rt[:], in_=bv("f_row"))

            # SBUF-resident transposed CWNN accumulators [D, REP]
            agg_do = cpool.tile([D, REP], F16, tag="agg_do")
            agg_up = cpool.tile([D, REP], F16, tag="agg_up")

            # transposed local x: hT [D, RNP]
            hT = hpool.tile([D, RNP], F16, tag="hT")
            for wv in range(RNP // WWIN):
                nc.scalar.dma_start_transpose(
                    hT[:, wv * WWIN:(wv + 1) * WWIN],
                    x_sh_ap[wv * WWIN:(wv + 1) * WWIN, :])

            # replicated tables via AllGather (collectives cannot read IO
            # tensors -> bounce through internal DRAM first)
            if stage >= 1:
                nc.sync.dma_start(out=x_bnc.ap(), in_=x_sh_ap)
                for ch in range(4):
                    r0 = ch * (REP // 4)
                    rn = (REP // 4) if ch < 3 else REP - 3 * (REP // 4)
                    nc.sync.dma_start(out=xe_bnc.ap()[r0:r0 + rn, :],
                                      in_=xe_sh_ap[r0:r0 + rn, :])
                nc.gpsimd.collective_compute(
                    "AllGather", mybir.AluOpType.bypass, replica_groups=rg,
                    ins=[x_bnc.ap().opt()], outs=[x16.ap().opt()])
                nc.gpsimd.collective_compute(
                    "AllGather", mybir.AluOpType.bypass, replica_groups=rg,
                    ins=[xe_bnc.ap().opt()], outs=[xe16.ap().opt()])

            # ============ helper: grid-S aggregation ============
            def grid_s_agg(agg, n_win, offs, idx_tile, row_tile, tables,
                           phase):
                """agg: SBUF tile [128, n_win*128]; tables: list of
                (dram_tensor, row_lo, row_n, tok_lo, tok_hi) gather sources."""
                for (src, row_lo, row_n, t0, t1) in tables:
                    t = t0
                    while t < t1:
                        n = min(MAXTOK, t1 - t)
                        G = n // 128
                        g = gpool.tile([128, MAXTOK // 128, D], F16,
                                       tag="g")
                        nc.gpsimd.dma_gather(
                            g[:, :G, :], src.ap()[row_lo:row_lo + row_n, :],
                            idx_tile[:, t // 16:(t + n) // 16], n, n, D)
                        # one batched S build for all G chunks of this gather
                        stb = spool.tile([128, MAXTOK // 128, GWIN], F16,
                                         tag="s1h")
                        nc.vector.tensor_tensor(
                            out=stb[:, :G, :],
                            in0=iota[:].unsqueeze(1)
                                .broadcast_to([128, G, GWIN]),
                            in1=row_tile[:, t // 128:(t + n) // 128]
                                .unsqueeze(2).broadcast_to([128, G, GWIN]),
                            op=EQ)
                        for c in range(G):
                            tok = t + c * 128
                            ki = int(np.searchsorted(offs, tok, side="right")) - 1
                            wi = ki % n_win if phase == "f" else ki
                            ps = pps.tile([128, GWIN], F32, tag="ps_s")
                            nc.tensor.matmul(
                                ps[:], lhsT=g[:, c, :], rhs=stb[:, c, :],
                                start=True, stop=True)
                            nc.vector.tensor_tensor(
                                out=agg[:, wi * GWIN:(wi + 1) * GWIN],
                                in0=agg[:, wi * GWIN:(wi + 1) * GWIN],
                                in1=ps[:], op=mybir.AluOpType.add)
                        t += n

            # ============ interleaved layers ============
            n_layers = L if stage >= 5 else (1 if stage >= 2 else 0)
            CWWIN = p.CWWIN
            for i in range(n_layers):
                # ---------- CWNN layer i ----------
                he_table = xe16 if i == 0 else he_full[i]
                heT_nxt = heT_d[i % 2]

                nc.vector.memset(agg_do[:], 0.0)
                nc.vector.memset(agg_up[:], 0.0)

                # gather ranges are contiguous per (lap, bucket); chunks map
                # statically to (lap, b, window) cells via cw_offs
                coffs = p.cw_offs
                lb_ranges = []
                for lap in range(2):
                    for bb in range(NBUCK_E):
                        k0 = (lap * NBUCK_E + bb) * (REP // CWWIN)
                        k1 = k0 + (REP // CWWIN)
                        t0, t1 = int(coffs[k0]), int(coffs[k1])
                        if t1 > t0:
                            lb_ranges.append((bb, t0, t1))
                for (bb, t0, t1) in lb_ranges:
                    row_lo = bb * BUCK
                    row_n = min(BUCK, EP_TBL - row_lo)
                    t = t0
                    while t < t1:
                        n = min(MAXTOK, t1 - t)
                        G = n // 128
                        g = gpool.tile([128, MAXTOK // 128, D], F16, tag="g")
                        nc.gpsimd.dma_gather(
                            g[:, :G, :], he_table.ap()[row_lo:row_lo + row_n, :],
                            cw_git[:, t // 16:(t + n) // 16], n, n, D)
                        nc.vector.tensor_tensor(
                            out=g[:, :G, :], in0=g[:, :G, :],
                            in1=cw_vt[:, t // 128:(t + n) // 128]
                                .unsqueeze(2).broadcast_to([128, G, D]),
                            op=mybir.AluOpType.mult)
                        # one batched S build for all G chunks of this gather
                        stb = spool.tile([128, MAXTOK // 128, CWWIN], F16,
                                         tag="s2h")
                        nc.vector.tensor_tensor(
                            out=stb[:, :G, :],
                            in0=iota256[:].unsqueeze(1)
                                .broadcast_to([128, G, CWWIN]),
                            in1=cw_rt[:, t // 128:(t + n) // 128]
                                .unsqueeze(2).broadcast_to([128, G, CWWIN]),
                            op=EQ)
                        for c in range(G):
                            tok = t + c * 128
                            ki = int(np.searchsorted(coffs, tok,
                                                     side="right")) - 1
                            lap, _, ww = p.cw_keys[ki]
                            agg = agg_do if lap == 0 else agg_up
                            ps = ppc.tile([128, CWWIN], F32, tag="ps_c")
                            nc.tensor.matmul(ps[:], lhsT=g[:, c, :],
                                             rhs=stb[:, c, :],
                                             start=True, stop=True)
                            nc.vector.tensor_tensor(
                                out=agg[:, ww * CWWIN:(ww + 1) * CWWIN],
                                in0=agg[:, ww * CWWIN:(ww + 1) * CWWIN],
                                in1=ps[:], op=mybir.AluOpType.add)
                        t += n

                if stage < 3:
                    break
                # W phase: he_next = relu(he@W0 + acc_do@W1 + acc_up@W2)
                # (agg_do/agg_up already live in SBUF, transposed)
                for w in range(REP // WWIN):
                    c0 = w * WWIN
                    cn = WWIN
                    hw = wpool.tile([128, WWIN], F16, tag="hw")
                    if i == 0:
                        nc.scalar.dma_start_transpose(
                            hw[:, :cn], xe_sh_ap[c0:c0 + cn, :])
                    else:
                        nc.sync.dma_start(
                            out=hw[:, :cn],
                            in_=heT_d[(i + 1) % 2].ap()[:, c0:c0 + cn])
                    ps = ppw.tile([128, WWIN], F32, tag="ps_w")
                    nc.tensor.matmul(ps[:, :cn], lhsT=w0[:, i * D:(i + 1) * D],
                                     rhs=hw[:, :cn], start=True, stop=False)
                    nc.tensor.matmul(ps[:, :cn], lhsT=w1[:, i * D:(i + 1) * D],
                                     rhs=agg_do[:, c0:c0 + cn],
                                     start=False, stop=False)
                    nc.tensor.matmul(ps[:, :cn], lhsT=w2[:, i * D:(i + 1) * D],
                                     rhs=agg_up[:, c0:c0 + cn],
                                     start=False, stop=True)
                    hn = wpool.tile([128, WWIN], F16, tag="hn")
                    nc.scalar.activation(hn[:, :cn], ps[:, :cn], RELU)
                    nc.sync.dma_start(out=heT_nxt.ap()[:, c0:c0 + cn],
                                      in_=hn[:, :cn])
                    # row-major blocks for AllGather input via PE transpose
                    rows = tpool.tile([128, WWIN // 128, D], F16, tag="cw_rr")
                    for tt_i in range(WWIN // 128):
                        r0 = tt_i * 128
                        pst = pps.tile([128, D], F32, tag="ps_s")
                        nc.tensor.matmul(pst[:], lhsT=hn[:, r0:r0 + 128],
                                         rhs=ident[:], start=True, stop=True)
                        nc.scalar.activation(rows[:, tt_i, :], pst[:], COPY)
                    nc.sync.dma_start(
                        out=he_bounce[i].ap()[c0:c0 + cn, :]
                            .rearrange("(g q) d -> q g d", q=128),
                        in_=rows[:])

                nc.gpsimd.collective_compute(
                    "AllGather", mybir.AluOpType.bypass, replica_groups=rg,
                    ins=[he_bounce[i].ap().opt()],
                    outs=[he_full[i + 1].ap().opt()])

                if stage < 4:
                    break
                # ---------- GNN layer i ----------
                h_table = x16 if i == 0 else h_full[i]
                agg = apool.tile([128, RNP], F16, tag="gagg")
                nc.vector.memset(agg[:], 0.0)
                grid_s_agg(agg, NWIN_N, p.gnn_win_offs_np, gnn_it, gnn_rt,
                           [(h_table, 0, NP_TBL, 0, p.gnn_Tg)], "g")
                nc.vector.tensor_tensor(out=agg[:], in0=agg[:],
                                        in1=hT[:],
                                        op=mybir.AluOpType.add)
                hT = hpool.tile([D, RNP], F16, tag="hT")
                for w in range(_ceil(RNP, WWIN)):
                    c0 = w * WWIN
                    cn = min(WWIN, RNP - c0)
                    ps = ppw.tile([128, WWIN], F32, tag="ps_w")
                    nc.tensor.matmul(ps[:, :cn], lhsT=gW[:, i * D:(i + 1) * D],
                                     rhs=agg[:, c0:c0 + cn],
                                     start=True, stop=True)
                    nc.scalar.activation(hT[:, c0:c0 + cn], ps[:, :cn], RELU)
                if i < L - 1:
                    for t in range(RNP // 128):
                        r0 = t * 128
                        tt = tpool.tile([128, 128], F16, tag="g_tt")
                        nc.scalar.dma_start_transpose(
                            tt[:], hT[:, r0:r0 + 128])
                        nc.sync.dma_start(
                            out=h_bounce[i].ap()[r0:r0 + 128, :],
                            in_=tt[:])
                    nc.gpsimd.collective_compute(
                        "AllGather", mybir.AluOpType.bypass, replica_groups=rg,
                        ins=[h_bounce[i].ap().opt()],
                        outs=[h_full[i + 1].ap().opt()])

            # ============ final: xed = segsum(he3, row) + segsum(he3, col) ==
            fagg = apool.tile([128, RNP], F16, tag="fagg")
            nc.vector.memset(fagg[:], 0.0)
            ftables = []
            for bi, bb in enumerate(range(NBUCK_E) if stage >= 6 else []):
                klo = bi * NWIN_N
                t0 = int(p.f_offs[klo])
                t1 = int(p.f_offs[klo + NWIN_N])
                row_lo = bb * BUCK
                row_n = min(BUCK, EP_TBL - row_lo)
                if t1 > t0:
                    ftables.append((he_full[3], row_lo, row_n, t0, t1))
            grid_s_agg(fagg, NWIN_N, p.f_offs, f_it, f_rt, ftables, "f")

            # output: [RNP, 0:128] = h3 rows, [RNP, 128:256] = xed
            for t in range(NWIN_N):
                c0 = t * 128
                psx = pps.tile([128, 128], F32, tag="ps_s")
                nc.tensor.matmul(psx[:], lhsT=hT[:, c0:c0 + 128], rhs=ident[:],
                                 start=True, stop=True)
                ox = tpool.tile([128, 128], F16, tag="tt16")
                nc.vector.tensor_copy(ox[:], psx[:])
                nc.sync.dma_start(out=out.ap()[c0:c0 + 128, 0:D], in_=ox[:])
                psy = pps.tile([128, 128], F32, tag="ps_s")
                nc.tensor.matmul(psy[:], lhsT=fagg[:, c0:c0 + 128],
                                 rhs=ident[:], start=True, stop=True)
                oy = tpool.tile([128, 128], F16, tag="tt16")
                nc.vector.tensor_copy(oy[:], psy[:])
                nc.sync.dma_start(out=out.ap()[c0:c0 + 128, D:2 * D], in_=oy[:])

    nc.compile()
    return nc


# ---------------------------------------------------------------------------
# PJRT runner (axon path; no /dev/neuron* on client)
# ---------------------------------------------------------------------------

def _make_runner(nc):
    import jax
    import time
    from jax.sharding import Mesh, PartitionSpec
    from jax.experimental.shard_map import shard_map
    import concourse.mybir as mybir
    import concourse.bass2jax as bass2jax
    from concourse.bass2jax import _bass_exec_p, install_neuronx_cc_hook

    install_neuronx_cc_hook()
    partition_name = nc.partition_id_tensor.name if nc.partition_id_tensor else None

    in_names, out_names, out_avals, zero_outs = [], [], [], []
    for alloc in nc.m.functions[0].allocations:
        if not isinstance(alloc, mybir.MemoryLocationSet):
            continue
        name = alloc.memorylocations[0].name
        if alloc.kind == "ExternalInput":
            if name != partition_name:
                in_names.append(name)
        elif alloc.kind == "ExternalOutput":
            out_names.append(name)
            shape = tuple(alloc.tensor_shape)
            dtype = mybir.dt.np(alloc.dtype)
            out_avals.append(jax.core.ShapedArray(shape, dtype))
            zero_outs.append(np.zeros(shape, dtype))
    n_params = len(in_names)
    all_in_names = list(in_names) + list(out_names)
    if partition_name is not None:
        all_in_names.append(partition_name)

    def _body(*args):
        operands = list(args)
        if partition_name is not None:
            operands.append(bass2jax.partition_id_tensor())
        outs = _bass_exec_p.bind(
            *operands,
            out_avals=tuple(out_avals),
            in_names=tuple(all_in_names),
            out_names=tuple(out_names),
            lowering_input_output_aliases=(),
            sim_require_finite=True,
            sim_require_nnan=True,
            nc=nc,
        )
        return tuple(outs)

    devices = jax.devices()[:NCORES]
    mesh = Mesh(np.asarray(devices), ("core",))
    in_specs = (PartitionSpec("core"),) * (n_params + len(out_names))
    out_specs = (PartitionSpec("core"),) * len(out_names)
    sharded = jax.jit(
        shard_map(_body, mesh=mesh, in_specs=in_specs, out_specs=out_specs,
                  check_rep=False),
        keep_unused=True,
    )

    def run_fn(in_maps, iters=1):
        per_core = [[np.asarray(m[name]) for name in in_names] for m in in_maps]
        concat_in = [np.concatenate([per_core[c][i] for c in range(NCORES)], axis=0)
                     for i in range(n_params)]
        concat_zeros = [np.zeros((NCORES * z.shape[0], *z.shape[1:]), z.dtype)
                        for z in zero_outs]
        dev_in = [jax.device_put(a) for a in concat_in]
        dev_zero = [jax.device_put(z) for z in concat_zeros]
        out = sharded(*dev_in, *dev_zero)
        jax.block_until_ready(out)
        t0 = time.perf_counter()
        if iters > 1:
            for _ in range(iters):
                out = sharded(*dev_in, *dev_zero)
            jax.block_until_ready(out)
            dt = (time.perf_counter() - t0) / iters
        else:
            dt = 0.0
        results = [
            {name: np.asarray(out[i]).reshape(NCORES, *out_avals[i].shape)[c]
             for i, name in enumerate(out_names)}
            for c in range(NCORES)
        ]
        return results, dt

    return run_fn

# ---------------------------------------------------------------------------

_CACHE = {}


def _get_runner(p):
    key = p.skey
    if key in _CACHE:
        return _CACHE[key]
    # np arrays needed by builder
    offs = np.zeros(NWIN_N + 1, np.int64)
    offs[1:] = np.cumsum(p.gnn_win_slots)
    p.gnn_win_offs_np = offs
    nc = _build(p)
    run_fn = _make_runner(nc)
    _CACHE[key] = run_fn
    return run_fn


def kernel(**inputs):
    p = _preprocess(**inputs)
    run_fn = _get_runner(p)
    in_maps = [{"blob": p.blob[r]} for r in range(NCORES)]
    results, dt = run_fn(in_maps, iters=1)
    kernel.last_dt = dt
    kernel.run_fn = run_fn
    kernel.in_maps = in_maps
    outs = [results[r]["out"][:RN] for r in range(NCORES)]
    return np.concatenate(outs, axis=0).astype(np.float32)


# revision 32
# speedup vs baseline: 1.2288x; 1.0799x over previous
"""Trainium2 Bass kernel for nn_CellNetwork (GNN + CWNN message passing).

Self-contained: takes FULL inputs, shards across 8 NeuronCores internally,
returns FULL output [20000, 256] fp32.

Strategy (SPMD, one program for all 8 ranks; per-rank data differs but all
instruction shapes are rank-uniform via max-over-ranks padding):
  - fp16 on device, fp32 PSUM accumulation; fp16 output (host converts).
  - Host->device traffic is minimized (the PJRT/axon path costs ~1.6 ms per
    input tensor per execute): everything ships as ONE packed int16 blob per
    rank -- per-rank feature shards + compact int16 token indices + 2-byte
    row/val tables. Replicated gather tables are built on device via
    AllGather (Shared outputs); one-hot S matrices are built on the fly from
    row indices (iota is_equal on DVE).
  - GNN over nodes: sharded by node (2500/rank). Segment-sum via "S-matrix"
    PE matmuls on a fixed grid of 128-row windows; h tables re-replicated
    per layer via AllGather.
  - CWNN over cells: sharded by cell (25000/rank). Messages gathered from a
    replicated he table (dma_gather, int16 indices -> 7 x 32768-row buckets),
    scaled by val (DVE broadcast mult), then segment-summed by S-matmul:
    tokens sorted by (laplacian, bucket, 256-row window), one matmul per
    128-token chunk into a PSUM window, added into SBUF-resident transposed
    accumulators agg_do/agg_up [128, 25088] (duplicate rows sum in the PE --
    no scatter-add, no write races). The W phase consumes the aggs directly
    (already transposed); row-major he for the next AllGather is produced by
    PE transpose + one strided DMA per 512 rows.
  - Final dual scatter-add of cells to nodes: grid-S matmuls over he3.
"""
import sys
import numpy as np

sys.path.insert(0, "/opt/trn_rl_repo")

N = 20000
E = 200000
D = 128
NNZ = 400000
L = 3
NCORES = 8
RN = N // NCORES          # 2500 nodes per rank
RE = E // NCORES          # 25000 cells per rank
BUCK = 32768              # gather index window (int16 limit)
NBUCK_E = (NCORES * 25088 + BUCK - 1) // BUCK   # 7 (on padded table)
GWIN = 128                # grid-S window (rows per PSUM tile)
NWIN_N = (RN + GWIN - 1) // GWIN   # 20 windows for 2500 local nodes
RNP = NWIN_N * GWIN       # 2560 padded local nodes
WWIN = 512                # W-phase window (cols per matmul)
MAXTOK = 1024             # max tokens per dma_gather/scatter (SWDGE ring = 1024 descs)
REP = 25088               # cells per rank, padded to 49*512
EP_TBL = NCORES * REP     # 200704-row padded cell table
NP_TBL = NCORES * RNP     # 20480-row padded node table


def _pad128(n):
    return (n + 127) & ~127


def _ceil(a, b):
    return (a + b - 1) // b


# ---------------------------------------------------------------------------
# Host-side preprocessing
# ---------------------------------------------------------------------------

def _idx16(tokens, n_slots):
    """[16, n_slots/16] int16: token t at (t%16, t//16). Expanded to 128
    partitions (x8 replication) on device."""
    W = n_slots // 16
    arr = np.zeros((16, W), np.int16)
    t = np.arange(len(tokens))
    arr[t % 16, t // 16] = tokens.astype(np.int16)
    return arr


def _val_tile(vals, n_slots):
    """[128, n_slots/128] fp16: token t at (t%128, t//128)."""
    G = n_slots // 128
    arr = np.zeros((128, G), np.float16)
    t = np.arange(len(vals))
    arr[t % 128, t // 128] = vals.astype(np.float16)
    return arr


def _row_tile(rows, n_slots):
    """[128, n_slots/128] fp16: window-row of token t at (t%128, t//128);
    padded slots get -1 (matches no iota value -> zero S column)."""
    G = n_slots // 128
    arr = np.full((128, G), -1.0, np.float16)
    t = np.arange(len(rows))
    arr[t % 128, t // 128] = rows.astype(np.float16)
    return arr


def _prep_grid_s(per_rank_tokens, n_cells_keys):
    """Grid-S packing: tokens grouped into cells (ordered by key tuple list
    n_cells_keys); each cell padded to max-over-ranks chunk count."""
    nr = len(per_rank_tokens)
    cell_slots = []
    for key in n_cells_keys:
        mx = 0
        for r in range(nr):
            t = per_rank_tokens[r].get(key)
            if t is not None:
                mx = max(mx, len(t[0]))
        cell_slots.append(_pad128(mx))
    total = int(np.sum(cell_slots))
    offs = np.zeros(len(cell_slots) + 1, np.int64)
    offs[1:] = np.cumsum(cell_slots)
    return cell_slots, offs, total


class _Prep:
    pass


def _preprocess(x, xe, gnn_W, cwnn_W0, cwnn_W1, cwnn_W2, Ldo_val, Lup_val,
                edges, row, col, Ldo_idx, Lup_idx):
    p = _Prep()
    f16 = np.float16
    x16 = np.asarray(x, np.float32).astype(f16)
    xe16 = np.asarray(xe, np.float32).astype(f16)
    # per-rank padded shards
    p.x_sh = []
    p.xe_sh = []
    for r in range(NCORES):
        xs = np.zeros((RNP, D), f16)
        xs[:RN] = x16[r * RN:(r + 1) * RN]
        p.x_sh.append(xs)
        xes = np.zeros((REP, D), f16)
        xes[:RE] = xe16[r * RE:(r + 1) * RE]
        p.xe_sh.append(xes)
    p.gnnW = np.ascontiguousarray(
        np.asarray(gnn_W, np.float32).astype(f16).transpose(1, 0, 2).reshape(D, L * D))
    p.W0 = np.ascontiguousarray(
        np.asarray(cwnn_W0, np.float32).astype(f16).transpose(1, 0, 2).reshape(D, L * D))
    p.W1 = np.ascontiguousarray(
        np.asarray(cwnn_W1, np.float32).astype(f16).transpose(1, 0, 2).reshape(D, L * D))
    p.W2 = np.ascontiguousarray(
        np.asarray(cwnn_W2, np.float32).astype(f16).transpose(1, 0, 2).reshape(D, L * D))
    p.ident = np.eye(D, dtype=f16)
    p.iota = np.tile(np.arange(128, dtype=f16)[None, :], (128, 1))
    p.iota256 = np.tile(np.arange(256, dtype=f16)[None, :], (128, 1))

    edges = np.asarray(edges, np.int64)
    row = np.asarray(row, np.int64)
    col = np.asarray(col, np.int64)
    Ldo_idx = np.asarray(Ldo_idx, np.int64)
    Lup_idx = np.asarray(Lup_idx, np.int64)

    # ---------------- GNN tokens (grid-S, 1 bucket, 20 windows) -----------
    src, dst = edges[0], edges[1]
    rk = dst // RN
    per_rank = []
    for r in range(NCORES):
        sel = rk == r
        s_src = src[sel]
        s_src = (s_src // RN) * RNP + s_src % RN
        s_row = dst[sel] - r * RN
        w = s_row // GWIN
        order = np.argsort(w, kind="stable")
        s_src, s_row, w = s_src[order], s_row[order], w[order]
        d = {}
        for wi in range(NWIN_N):
            m = w == wi
            d[wi] = (s_src[m], s_row[m], None)
        per_rank.append(d)
    keys = list(range(NWIN_N))
    cs, offs, Tg = _prep_grid_s(per_rank, keys)
    p.gnn_win_slots = cs          # slots per window (multiple of 128)
    p.gnn_Tg = Tg
    p.gnn_idx = []
    p.gnn_row = []
    for r in range(NCORES):
        toks = np.zeros(Tg, np.int64)
        rows_all = np.full(Tg, -1, np.int64)
        for wi in range(NWIN_N):
            idx, rows, _ = per_rank[r][wi]
            o = offs[wi]
            n = len(idx)
            toks[o:o + n] = idx
            rows_all[o:o + n] = rows - wi * GWIN
        p.gnn_idx.append(_idx16(toks, Tg))
        p.gnn_row.append(_row_tile(rows_all, Tg))

    # ---------------- CWNN tokens (lap x bucket x 256-row window grid) -----
    # token: (col_global, row_local, val) ; lap 0 = Ldo, 1 = Lup
    # Receive side is S-matmul into SBUF agg (no dma_scatter_add): tokens
    # sorted by (lap, bucket, window); each cell padded to max-over-ranks
    # (multiple of 128); duplicates within a chunk are fine (PE sums them).
    lap_data = [(Ldo_idx, np.asarray(Ldo_val)), (Lup_idx, np.asarray(Lup_val))]
    CWWIN = 256
    NWIN_E = REP // CWWIN     # 98
    cell_tokens = [dict() for _ in range(NCORES)]  # key (lap, b, w)
    for lap, (lidx, lval) in enumerate(lap_data):
        lrow, lcol = lidx[0], lidx[1]
        rr = lrow // RE
        for r in range(NCORES):
            sel = rr == r
            c_g = lcol[sel]
            c_g = (c_g // RE) * REP + c_g % RE
            r_l = lrow[sel] - r * RE
            v = lval[sel]
            b = c_g // BUCK
            w = r_l // CWWIN
            key = b * NWIN_E + w
            order = np.argsort(key, kind="stable")
            c_g, r_l, v, key = c_g[order], r_l[order], v[order], key[order]
            for bb in range(NBUCK_E):
                for ww in range(NWIN_E):
                    m = key == bb * NWIN_E + ww
                    if m.any():
                        cell_tokens[r][(lap, bb, ww)] = (
                            c_g[m] - bb * BUCK, r_l[m] - ww * CWWIN, v[m])
    cw_keys = [(lap, bb, ww) for lap in range(2) for bb in range(NBUCK_E)
               for ww in range(NWIN_E)]
    cw_slots = []
    for key in cw_keys:
        mx = 0
        for r in range(NCORES):
            t = cell_tokens[r].get(key)
            if t is not None:
                mx = max(mx, len(t[0]))
        cw_slots.append(_pad128(mx))
    Tc = int(np.sum(cw_slots))
    coffs = np.zeros(len(cw_slots) + 1, np.int64)
    coffs[1:] = np.cumsum(cw_slots)
    p.cw_keys = cw_keys
    p.cw_slots = cw_slots
    p.cw_offs = coffs
    p.cw_Tc = Tc
    p.CWWIN = CWWIN
    p.cw_gidx = []
    p.cw_row = []
    p.cw_val = []
    for r in range(NCORES):
        gt = np.zeros(Tc, np.int64)
        rt = np.full(Tc, -1, np.int64)
        vt = np.zeros(Tc, np.float32)
        for si, key in enumerate(cw_keys):
            t = cell_tokens[r].get(key)
            if t is None:
                continue
            o = coffs[si]
            n = len(t[0])
            gt[o:o + n] = t[0]
            rt[o:o + n] = t[1]
            vt[o:o + n] = t[2]
        p.cw_gidx.append(_idx16(gt, Tc))
        p.cw_row.append(_row_tile(rt, Tc))
        p.cw_val.append(_val_tile(vt, Tc))

    # ---------------- final tokens (bucket x window grid-S) ----------------
    nodes = np.concatenate([row, col])
    cells = np.concatenate([np.arange(E), np.arange(E)])
    rk = nodes // RN
    per_rank_f = []
    for r in range(NCORES):
        sel = rk == r
        cc = cells[sel]
        cc = (cc // RE) * REP + cc % RE
        rl = nodes[sel] - r * RN
        b = cc // BUCK
        w = rl // GWIN
        d = {}
        for bb in range(NBUCK_E):
            for wi in range(NWIN_N):
                m = (b == bb) & (w == wi)
                d[(bb, wi)] = (cc[m] - bb * BUCK, rl[m], None)
        per_rank_f.append(d)
    fkeys = [(bb, wi) for bb in range(NBUCK_E) for wi in range(NWIN_N)]
    fcs, foffs, Tf = _prep_grid_s(per_rank_f, fkeys)
    p.f_keys = fkeys
    p.f_slots = fcs
    p.f_offs = foffs
    p.f_Tf = Tf
    p.f_idx = []
    p.f_row = []
    for r in range(NCORES):
        toks = np.zeros(Tf, np.int64)
        rows_all = np.full(Tf, -1, np.int64)
        for ki, key in enumerate(fkeys):
            idx, rows, _ = per_rank_f[r][key]
            o = foffs[ki]
            n = len(idx)
            toks[o:o + n] = idx
            rows_all[o:o + n] = rows - key[1] * GWIN
        p.f_idx.append(_idx16(toks, Tf))
        p.f_row.append(_row_tile(rows_all, Tf))

    # ---------------- pack everything into one blob per rank --------------
    # (the PJRT/axon path has ~1.6ms per-input-tensor overhead per execute;
    # bytes are nearly free -> ship ONE int16 tensor per rank)
    p.blob_layout = _blob_layout(Tg, Tc, Tf)
    tot = p.blob_layout["_total"]
    p.blob = []
    for r in range(NCORES):
        parts = {
            "x_sh": p.x_sh[r], "xe_sh": p.xe_sh[r],
            "gnnW": p.gnnW, "W0": p.W0, "W1": p.W1, "W2": p.W2,
            "ident": p.ident, "iota": p.iota, "iota256": p.iota256,
            "gnn_idx": p.gnn_idx[r], "gnn_row": p.gnn_row[r],
            "cw_gidx": p.cw_gidx[r], "cw_row": p.cw_row[r],
            "cw_val": p.cw_val[r],
            "f_idx": p.f_idx[r], "f_row": p.f_row[r],
        }
        b = np.empty(tot, np.int16)
        for name, spec in p.blob_layout.items():
            if name == "_total":
                continue
            off, shape, _ = spec
            a = parts[name]
            n = a.size
            b[off:off + n] = np.ascontiguousarray(a).view(np.int16).ravel()
        p.blob.append(b)

    # structure key for compile cache
    p.skey = (Tg, tuple(cs), Tc, tuple(cw_slots), Tf, tuple(fcs))
    return p


def _blob_layout(Tg, Tc, Tf):
    """name -> (element offset, shape, dtype_tag); all 2-byte elements."""
    specs = [
        ("x_sh", (RNP, D), "f16"),
        ("xe_sh", (REP, D), "f16"),
        ("gnnW", (D, L * D), "f16"),
        ("W0", (D, L * D), "f16"),
        ("W1", (D, L * D), "f16"),
        ("W2", (D, L * D), "f16"),
        ("ident", (D, D), "f16"),
        ("iota", (D, D), "f16"),
        ("iota256", (D, 2 * D), "f16"),
        ("gnn_idx", (16, Tg // 16), "i16"),
        ("gnn_row", (128, Tg // 128), "f16"),
        ("cw_gidx", (16, Tc // 16), "i16"),
        ("cw_row", (128, Tc // 128), "f16"),
        ("cw_val", (128, Tc // 128), "f16"),
        ("f_idx", (16, Tf // 16), "i16"),
        ("f_row", (128, Tf // 128), "f16"),
    ]
    layout = {}
    off = 0
    for name, shape, tag in specs:
        n = int(np.prod(shape))
        layout[name] = (off, shape, tag)
        off += n
    layout["_total"] = off
    return layout


# ---------------------------------------------------------------------------
# Device program
# ---------------------------------------------------------------------------

def _build(p, stage=6):
    # stage (cumulative, for HW-time bisection; 6 = full kernel):
    # 0 const loads+out, 1 +table AGs, 2 +CWNN L0 gather/scatter,
    # 3 +W phase L0 + he AG, 4 +GNN L0, 5 +all layers, 6 +final
    import concourse.bacc as bacc
    import concourse.tile as tile
    import concourse.mybir as mybir
    F16 = mybir.dt.float16
    F32 = mybir.dt.float32
    I16 = mybir.dt.int16
    RELU = mybir.ActivationFunctionType.Relu
    COPY = mybir.ActivationFunctionType.Copy
    EQ = mybir.AluOpType.is_equal

    nc = bacc.Bacc("TRN2", target_bir_lowering=False, debug=False,
                   num_devices=NCORES)

    # ---- I/O: ONE packed int16 blob per rank (see _blob_layout) ----
    layout = p.blob_layout
    blob = nc.dram_tensor("blob", [layout["_total"]], I16,
                          kind="ExternalInput")

    def bv(name):
        off, shape, tag = layout[name]
        n = int(np.prod(shape))
        ap = blob.ap()[off:off + n].rearrange("(a b) -> a b", b=shape[1])
        return ap if tag == "i16" else ap.bitcast(F16)

    x_sh_ap = bv("x_sh")
    xe_sh_ap = bv("xe_sh")
    out = nc.dram_tensor("out", [RNP, 2 * D], F16, kind="ExternalOutput")

    # ---- internal DRAM ----
    x16 = nc.dram_tensor("x16_d", [NP_TBL, D], F16, addr_space="Shared")
    xe16 = nc.dram_tensor("xe16_d", [EP_TBL, D], F16, addr_space="Shared")
    h_full = [None,
              nc.dram_tensor("h_full1", [NP_TBL, D], F16, addr_space="Shared"),
              nc.dram_tensor("h_full2", [NP_TBL, D], F16, addr_space="Shared")]
    he_full = [None,
               nc.dram_tensor("he_full1", [EP_TBL, D], F16, addr_space="Shared"),
               nc.dram_tensor("he_full2", [EP_TBL, D], F16, addr_space="Shared"),
               nc.dram_tensor("he_full3", [EP_TBL, D], F16, addr_space="Shared")]
    h_bounce = [nc.dram_tensor(f"h_bounce{i}", [RNP, D], F16) for i in range(2)]
    he_bounce = [nc.dram_tensor(f"he_bounce{i}", [REP, D], F16)
                 for i in range(3)]
    heT_d = [nc.dram_tensor(f"heT{i}", [D, REP], F16) for i in range(2)]
    x_bnc = nc.dram_tensor("x_bnc", [RNP, D], F16)
    xe_bnc = nc.dram_tensor("xe_bnc", [REP, D], F16)

    rg = [list(range(NCORES))]

    with tile.TileContext(nc) as tc:
        with tc.tile_pool(name="const", bufs=1) as cpool, \
             tc.tile_pool(name="gat", bufs=3) as gpool, \
             tc.tile_pool(name="sbld", bufs=2) as spool, \
             tc.tile_pool(name="hTp", bufs=2) as hpool, \
             tc.tile_pool(name="wph", bufs=3) as wpool, \
             tc.tile_pool(name="agg", bufs=2) as apool, \
             tc.tile_pool(name="tt", bufs=4) as tpool, \
             tc.tile_pool(name="ps_s", bufs=2, space="PSUM") as pps, \
             tc.tile_pool(name="ps_c", bufs=4, space="PSUM") as ppc, \
             tc.tile_pool(name="ps_w", bufs=2, space="PSUM") as ppw:

            ident = cpool.tile([D, D], F16, tag="ident")
            nc.sync.dma_start(out=ident[:], in_=bv("ident"))
            iota = cpool.tile([D, D], F16, tag="iota")
            nc.sync.dma_start(out=iota[:], in_=bv("iota"))
            gW = cpool.tile([D, L * D], F16, tag="gW")
            nc.sync.dma_start(out=gW[:], in_=bv("gnnW"))
            w0 = cpool.tile([D, L * D], F16, tag="w0")
            nc.sync.dma_start(out=w0[:], in_=bv("W0"))
            w1 = cpool.tile([D, L * D], F16, tag="w1")
            nc.sync.dma_start(out=w1[:], in_=bv("W1"))
            w2 = cpool.tile([D, L * D], F16, tag="w2")
            nc.sync.dma_start(out=w2[:], in_=bv("W2"))

            # resident index tiles: expand [16, W] -> [128, W] (x8 stripes)
            def expand_idx(src_ap, W, tag):
                t = cpool.tile([128, W], I16, tag=tag)
                for j in range(8):
                    nc.sync.dma_start(out=t[16 * j:16 * (j + 1), :],
                                      in_=src_ap)
                return t

            iota256 = cpool.tile([D, 2 * D], F16, tag="iota256")
            nc.sync.dma_start(out=iota256[:], in_=bv("iota256"))
            gnn_it = expand_idx(bv("gnn_idx"), p.gnn_Tg // 16, "gnn_it")
            cw_git = expand_idx(bv("cw_gidx"), p.cw_Tc // 16, "cw_git")
            f_it = expand_idx(bv("f_idx"), p.f_Tf // 16, "f_it")
            cw_vt = cpool.tile([128, p.cw_Tc // 128], F16, tag="cw_vt")
            nc.sync.dma_start(out=cw_vt[:], in_=bv("cw_val"))
            cw_rt = cpool.tile([128, p.cw_Tc // 128], F16, tag="cw_rt")
            nc.sync.dma_start(out=cw_rt[:], in_=bv("cw_row"))
            gnn_rt = cpool.tile([128, p.gnn_Tg // 128], F16, tag="gnn_rt")
            nc.sync.dma_start(out=gnn_rt[:], in_=bv("gnn_row"))
            f_rt = cpool.tile([128, p.f_Tf // 128], F16, tag="f_rt")
            nc.sync.dma_start(out=f_rt[:], in_=bv("f_row"))

            # SBUF-resident transposed CWNN accumulators [D, REP]
            agg_do = cpool.tile([D, REP], F16, tag="agg_do")
            agg_up = cpool.tile([D, REP], F16, tag="agg_up")

            # transposed local x: hT [D, RNP]
            hT = hpool.tile([D, RNP], F16, tag="hT")
            for wv in range(RNP // WWIN):
                nc.scalar.dma_start_transpose(
                    hT[:, wv * WWIN:(wv + 1) * WWIN],
                    x_sh_ap[wv * WWIN:(wv + 1) * WWIN, :])

            # replicated tables via AllGather (collectives cannot read IO
            # tensors -> bounce through internal DRAM first)
            if stage >= 1:
                nc.sync.dma_start(out=x_bnc.ap(), in_=x_sh_ap)
                for ch in range(4):
                    r0 = ch * (REP // 4)
                    rn = (REP // 4) if ch < 3 else REP - 3 * (REP // 4)
                    nc.sync.dma_start(out=xe_bnc.ap()[r0:r0 + rn, :],
                                      in_=xe_sh_ap[r0:r0 + rn, :])
                nc.gpsimd.collective_compute(
                    "AllGather", mybir.AluOpType.bypass, replica_groups=rg,
                    ins=[x_bnc.ap().opt()], outs=[x16.ap().opt()])
                nc.gpsimd.collective_compute(
                    "AllGather", mybir.AluOpType.bypass, replica_groups=rg,
                    ins=[xe_bnc.ap().opt()], outs=[xe16.ap().opt()])

            # ============ helper: grid-S aggregation ============
            def grid_s_agg(agg, n_win, offs, idx_tile, row_tile, tables,
                           phase):
                """agg: SBUF tile [128, n_win*128]; tables: list of
                (dram_tensor, row_lo, row_n, tok_lo, tok_hi) gather sources."""
                for (src, row_lo, row_n, t0, t1) in tables:
                    t = t0
                    while t < t1:
                        n = min(MAXTOK, t1 - t)
                        G = n // 128
                        g = gpool.tile([128, MAXTOK // 128, D], F16,
                                       tag="g")
                        nc.gpsimd.dma_gather(
                            g[:, :G, :], src.ap()[row_lo:row_lo + row_n, :],
                            idx_tile[:, t // 16:(t + n) // 16], n, n, D)
                        # one batched S build for all G chunks of this gather
                        stb = spool.tile([128, MAXTOK // 128, GWIN], F16,
                                         tag="s1h")
                        nc.vector.tensor_tensor(
                            out=stb[:, :G, :],
                            in0=iota[:].unsqueeze(1)
                                .broadcast_to([128, G, GWIN]),
                            in1=row_tile[:, t // 128:(t + n) // 128]
                                .unsqueeze(2).broadcast_to([128, G, GWIN]),
                            op=EQ)
                        for c in range(G):
                            tok = t + c * 128
                            ki = int(np.searchsorted(offs, tok, side="right")) - 1
                            wi = ki % n_win if phase == "f" else ki
                            ps = pps.tile([128, GWIN], F32, tag="ps_s")
                            nc.tensor.matmul(
                                ps[:], lhsT=g[:, c, :], rhs=stb[:, c, :],
                                start=True, stop=True)
                            nc.vector.tensor_tensor(
                                out=agg[:, wi * GWIN:(wi + 1) * GWIN],
                                in0=agg[:, wi * GWIN:(wi + 1) * GWIN],
                                in1=ps[:], op=mybir.AluOpType.add)
                        t += n

            # ============ interleaved layers ============
            n_layers = L if stage >= 5 else (1 if stage >= 2 else 0)
            CWWIN = p.CWWIN
            for i in range(n_layers):
                # ---------- CWNN layer i ----------
                he_table = xe16 if i == 0 else he_full[i]
                heT_nxt = heT_d[i % 2]

                nc.vector.memset(agg_do[:], 0.0)
                nc.vector.memset(agg_up[:], 0.0)

                # gather ranges are contiguous per (lap, bucket); chunks map
                # statically to (lap, b, window) cells via cw_offs
                coffs = p.cw_offs
                lb_ranges = []
                for lap in range(2):
                    for bb in range(NBUCK_E):
                        k0 = (lap * NBUCK_E + bb) * (REP // CWWIN)
                        k1 = k0 + (REP // CWWIN)
                        t0, t1 = int(coffs[k0]), int(coffs[k1])
                        if t1 > t0:
                            lb_ranges.append((bb, t0, t1))
                for (bb, t0, t1) in lb_ranges:
                    row_lo = bb * BUCK
                    row_n = min(BUCK, EP_TBL - row_lo)
                    t = t0
                    while t < t1:
                        n = min(MAXTOK, t1 - t)
                        G = n // 128
                        g = gpool.tile([128, MAXTOK // 128, D], F16, tag="g")
                        nc.gpsimd.dma_gather(
                            g[:, :G, :], he_table.ap()[row_lo:row_lo + row_n, :],
                            cw_git[:, t // 16:(t + n) // 16], n, n, D)
                        nc.vector.tensor_tensor(
                            out=g[:, :G, :], in0=g[:, :G, :],
                            in1=cw_vt[:, t // 128:(t + n) // 128]
                                .unsqueeze(2).broadcast_to([128, G, D]),
                            op=mybir.AluOpType.mult)
                        # one batched S build for all G chunks of this gather
                        stb = spool.tile([128, MAXTOK // 128, CWWIN], F16,
                                         tag="s2h")
                        nc.vector.tensor_tensor(
                            out=stb[:, :G, :],
                            in0=iota256[:].unsqueeze(1)
                                .broadcast_to([128, G, CWWIN]),
                            in1=cw_rt[:, t // 128:(t + n) // 128]
                                .unsqueeze(2).broadcast_to([128, G, CWWIN]),
                            op=EQ)
                        for c in range(G):
                            tok = t + c * 128
                            ki = int(np.searchsorted(coffs, tok,
                                                     side="right")) - 1
                            lap, _, ww = p.cw_keys[ki]
                            agg = agg_do if lap == 0 else agg_up
                            ps = ppc.tile([128, CWWIN], F32, tag="ps_c")
                            nc.tensor.matmul(ps[:], lhsT=g[:, c, :],
                                             rhs=stb[:, c, :],
                                             start=True, stop=True)
                            nc.vector.tensor_tensor(
                                out=agg[:, ww * CWWIN:(ww + 1) * CWWIN],
                                in0=agg[:, ww * CWWIN:(ww + 1) * CWWIN],
                                in1=ps[:], op=mybir.AluOpType.add)
                        t += n

                if stage < 3:
                    break
                # W phase: he_next = relu(he@W0 + acc_do@W1 + acc_up@W2)
                # (agg_do/agg_up already live in SBUF, transposed)
                for w in range(REP // WWIN):
                    c0 = w * WWIN
                    cn = WWIN
                    hw = wpool.tile([128, WWIN], F16, tag="hw")
                    if i == 0:
                        nc.scalar.dma_start_transpose(
                            hw[:, :cn], xe_sh_ap[c0:c0 + cn, :])
                    else:
                        nc.sync.dma_start(
                            out=hw[:, :cn],
                            in_=heT_d[(i + 1) % 2].ap()[:, c0:c0 + cn])
                    ps = ppw.tile([128, WWIN], F32, tag="ps_w")
                    nc.tensor.matmul(ps[:, :cn], lhsT=w0[:, i * D:(i + 1) * D],
                                     rhs=hw[:, :cn], start=True, stop=False)
                    nc.tensor.matmul(ps[:, :cn], lhsT=w1[:, i * D:(i + 1) * D],
                                     rhs=agg_do[:, c0:c0 + cn],
                                     start=False, stop=False)
                    nc.tensor.matmul(ps[:, :cn], lhsT=w2[:, i * D:(i + 1) * D],
                                     rhs=agg_up[:, c0:c0 + cn],
                                     start=False, stop=True)
                    hn = wpool.tile([128, WWIN], F16, tag="hn")
                    nc.scalar.activation(hn[:, :cn], ps[:, :cn], RELU)
                    nc.sync.dma_start(out=heT_nxt.ap()[:, c0:c0 + cn],
                                      in_=hn[:, :cn])
                    # row-major blocks for AllGather input via PE transpose
                    rows = tpool.tile([128, WWIN // 128, D], F16, tag="cw_rr")
                    for tt_i in range(WWIN // 128):
                        r0 = tt_i * 128
                        pst = pps.tile([128, D], F32, tag="ps_s")
                        nc.tensor.matmul(pst[:], lhsT=hn[:, r0:r0 + 128],
                                         rhs=ident[:], start=True, stop=True)
                        nc.scalar.activation(rows[:, tt_i, :], pst[:], COPY)
                    nc.sync.dma_start(
                        out=he_bounce[i].ap()[c0:c0 + cn, :]
                            .rearrange("(g q) d -> q g d", q=128),
                        in_=rows[:])

                nc.gpsimd.collective_compute(
                    "AllGather", mybir.AluOpType.bypass, replica_groups=rg,
                    ins=[he_bounce[i].ap().opt()],
                    outs=[he_full[i + 1].ap().opt()])

                if stage < 4:
                    break
                # ---------- GNN layer i ----------
                h_table = x16 if i == 0 else h_full[i]
                agg = apool.tile([128, RNP], F16, tag="gagg")
                nc.vector.memset(agg[:], 0.0)
                grid_s_agg(agg, NWIN_N, p.gnn_win_offs_np, gnn_it, gnn_rt,
                           [(h_table, 0, NP_TBL, 0, p.gnn_Tg)], "g")
                nc.vector.tensor_tensor(out=agg[:], in0=agg[:],
                                        in1=hT[:],
                                        op=mybir.AluOpType.add)
                hT = hpool.tile([D, RNP], F16, tag="hT")
                for w in range(_ceil(RNP, WWIN)):
                    c0 = w * WWIN
                    cn = min(WWIN, RNP - c0)
                    ps = ppw.tile([128, WWIN], F32, tag="ps_w")
                    nc.tensor.matmul(ps[:, :cn], lhsT=gW[:, i * D:(i + 1) * D],
                                     rhs=agg[:, c0:c0 + cn],
                                     start=True, stop=True)
                    nc.scalar.activation(hT[:, c0:c0 + cn], ps[:, :cn], RELU)
                if i < L - 1:
                    for t in range(RNP // 128):
                        r0 = t * 128
                        tt = tpool.tile([128, 128], F16, tag="g_tt")
                        nc.scalar.dma_start_transpose(
                            tt[:], hT[:, r0:r0 + 128])
                        nc.sync.dma_start(
                            out=h_bounce[i].ap()[r0:r0 + 128, :],
                            in_=tt[:])
                    nc.gpsimd.collective_compute(
                        "AllGather", mybir.AluOpType.bypass, replica_groups=rg,
                        ins=[h_bounce[i].ap().opt()],
                        outs=[h_full[i + 1].ap().opt()])

            # ============ final: xed = segsum(he3, row) + segsum(he3, col) ==
            fagg = apool.tile([128, RNP], F16, tag="fagg")
            nc.vector.memset(fagg[:], 0.0)
            ftables = []
            for bi, bb in enumerate(range(NBUCK_E) if stage >= 6 else []):
                klo = bi * NWIN_N
                t0 = int(p.f_offs[klo])
                t1 = int(p.f_offs[klo + NWIN_N])
                row_lo = bb * BUCK
                row_n = min(BUCK, EP_TBL - row_lo)
                if t1 > t0:
                    ftables.append((he_full[3], row_lo, row_n, t0, t1))
            grid_s_agg(fagg, NWIN_N, p.f_offs, f_it, f_rt, ftables, "f")

            # output: [RNP, 0:128] = h3 rows, [RNP, 128:256] = xed
            for t in range(NWIN_N):
                c0 = t * 128
                psx = pps.tile([128, 128], F32, tag="ps_s")
                nc.tensor.matmul(psx[:], lhsT=hT[:, c0:c0 + 128], rhs=ident[:],
                                 start=True, stop=True)
                ox = tpool.tile([128, 128], F16, tag="tt16")
                nc.vector.tensor_copy(ox[:], psx[:])
                nc.sync.dma_start(out=out.ap()[c0:c0 + 128, 0:D], in_=ox[:])
                psy = pps.tile([128, 128], F32, tag="ps_s")
                nc.tensor.matmul(psy[:], lhsT=fagg[:, c0:c0 + 128],
                                 rhs=ident[:], start=True, stop=True)
                oy = tpool.tile([128, 128], F16, tag="tt16")
                nc.vector.tensor_copy(oy[:], psy[:])
                nc.sync.dma_start(out=out.ap()[c0:c0 + 128, D:2 * D], in_=oy[:])

    nc.compile()
    return nc


# ---------------------------------------------------------------------------
# PJRT runner (axon path; no /dev/neuron* on client)
# ---------------------------------------------------------------------------

def _make_runner(nc):
    import jax
    import time
    from jax.sharding import Mesh, PartitionSpec
    from jax.experimental.shard_map import shard_map
    import concourse.mybir as mybir
    import concourse.bass2jax as bass2jax
    from concourse.bass2jax import _bass_exec_p, install_neuronx_cc_hook

    install_neuronx_cc_hook()
    partition_name = nc.partition_id_tensor.name if nc.partition_id_tensor else None

    in_names, out_names, out_avals, zero_outs = [], [], [], []
    for alloc in nc.m.functions[0].allocations:
        if not isinstance(alloc, mybir.MemoryLocationSet):
            continue
        name = alloc.memorylocations[0].name
        if alloc.kind == "ExternalInput":
            if name != partition_name:
                in_names.append(name)
        elif alloc.kind == "ExternalOutput":
            out_names.append(name)
            shape = tuple(alloc.tensor_shape)
            dtype = mybir.dt.np(alloc.dtype)
            out_avals.append(jax.core.ShapedArray(shape, dtype))
            zero_outs.append(np.zeros(shape, dtype))
    n_params = len(in_names)
    all_in_names = list(in_names) + list(out_names)
    if partition_name is not None:
        all_in_names.append(partition_name)

    def _body(*args):
        operands = list(args)
        if partition_name is not None:
            operands.append(bass2jax.partition_id_tensor())
        outs = _bass_exec_p.bind(
            *operands,
            out_avals=tuple(out_avals),
            in_names=tuple(all_in_names),
            out_names=tuple(out_names),
            lowering_input_output_aliases=(),
            sim_require_finite=True,
            sim_require_nnan=True,
            nc=nc,
        )
        return tuple(outs)

    devices = jax.devices()[:NCORES]
    mesh = Mesh(np.asarray(devices), ("core",))
    in_specs = (PartitionSpec("core"),) * (n_params + len(out_names))
    out_specs = (PartitionSpec("core"),) * len(out_names)
    sharded = jax.jit(
        shard_map(_body, mesh=mesh, in_specs=in_specs, out_specs=out_specs,
                  check_rep=False),
        keep_unused=True,
    )

    def run_fn(in_maps, iters=1):
        per_core = [[np.asarray(m[name]) for name in in_names] for m in in_maps]
        concat_in = [np.concatenate([per_core[c][i] for c in range(NCORES)], axis=0)
                     for i in range(n_params)]
        concat_zeros = [np.zeros((NCORES * z.shape[0], *z.shape[1:]), z.dtype)
                        for z in zero_outs]
        dev_in = [jax.device_put(a) for a in concat_in]
        dev_zero = [jax.device_put(z) for z in concat_zeros]
        out = sharded(*dev_in, *dev_zero)
        jax.block_until_ready(out)
        t0 = time.perf_counter()
        if iters > 1:
            for _ in range(iters):
                out = sharded(*dev_in, *dev_zero)
            jax.block_until_ready(out)
            dt = (time.perf_counter() - t0) / iters
        else:
            dt = 0.0
        results = [
            {name: np.asarray(out[i]).reshape(NCORES, *out_avals[i].shape)[c]
             for i, name in enumerate(out_names)}
            for c in range(NCORES)
        ]
        return results, dt

    return run_fn

# ---------------------------------------------------------------------------

_CACHE = {}


def _get_runner(p):
    key = p.skey
    if key in _CACHE:
        return _CACHE[key]
    # np arrays needed by builder
    offs = np.zeros(NWIN_N + 1, np.int64)
    offs[1:] = np.cumsum(p.gnn_win_slots)
    p.gnn_win_offs_np = offs
    nc = _build(p)
    run_fn = _make_runner(nc)
    _CACHE[key] = run_fn
    return run_fn


def kernel(**inputs):
    p = _preprocess(**inputs)
    run_fn = _get_runner(p)
    in_maps = [{"blob": p.blob[r]} for r in range(NCORES)]
    results, dt = run_fn(in_maps, iters=1)
    kernel.last_dt = dt
    kernel.run_fn = run_fn
    kernel.in_maps = in_maps
    outs = [results[r]["out"][:RN] for r in range(NCORES)]
    return np.concatenate(outs, axis=0).astype(np.float32)


# revision 33
# speedup vs baseline: 1.3354x; 1.0868x over previous
"""Trainium2 Bass kernel for nn_CellNetwork (GNN + CWNN message passing).

Self-contained: takes FULL inputs, shards across 8 NeuronCores internally,
returns FULL output [20000, 256] fp32.

Strategy (SPMD, one program for all 8 ranks; per-rank data differs but all
instruction shapes are rank-uniform via max-over-ranks padding):
  - fp16 on device, fp32 PSUM accumulation; fp16 output (host converts).
  - Host->device traffic is minimized (the PJRT/axon path costs ~1.6 ms per
    input tensor per execute): everything ships as ONE packed int16 blob per
    rank -- per-rank feature shards + compact int16 token indices + 2-byte
    row/val tables. Replicated gather tables are built on device via
    AllGather (Shared outputs); one-hot S matrices are built on the fly from
    row indices (iota is_equal on DVE).
  - GNN over nodes: sharded by node (2500/rank). Segment-sum via "S-matrix"
    PE matmuls on a fixed grid of 128-row windows; h tables re-replicated
    per layer via AllGather.
  - CWNN over cells: sharded by cell (25000/rank). Messages gathered from a
    replicated he table (dma_gather, int16 indices -> 7 x 32768-row buckets),
    scaled by val (DVE broadcast mult), then segment-summed by S-matmul:
    tokens sorted by (laplacian, bucket, 256-row window), one matmul per
    128-token chunk into a PSUM window, added into SBUF-resident transposed
    accumulators agg_do/agg_up [128, 25088] (duplicate rows sum in the PE --
    no scatter-add, no write races). The W phase consumes the aggs directly
    (already transposed); row-major he for the next AllGather is produced by
    PE transpose + one strided DMA per 512 rows.
  - Final dual scatter-add of cells to nodes: grid-S matmuls over he3.
"""
import sys
import numpy as np

sys.path.insert(0, "/opt/trn_rl_repo")

N = 20000
E = 200000
D = 128
NNZ = 400000
L = 3
NCORES = 8
RN = N // NCORES          # 2500 nodes per rank
RE = E // NCORES          # 25000 cells per rank
BUCK = 32768              # gather index window (int16 limit)
NBUCK_E = (NCORES * 25088 + BUCK - 1) // BUCK   # 7 (on padded table)
GWIN = 128                # grid-S window (rows per PSUM tile)
NWIN_N = (RN + GWIN - 1) // GWIN   # 20 windows for 2500 local nodes
RNP = NWIN_N * GWIN       # 2560 padded local nodes
WWIN = 512                # W-phase window (cols per matmul)
MAXTOK = 1024             # max tokens per dma_gather/scatter (SWDGE ring = 1024 descs)
REP = 25088               # cells per rank, padded to 49*512
EP_TBL = NCORES * REP     # 200704-row padded cell table
NP_TBL = NCORES * RNP     # 20480-row padded node table


def _pad128(n):
    return (n + 127) & ~127


def _ceil(a, b):
    return (a + b - 1) // b


# ---------------------------------------------------------------------------
# Host-side preprocessing
# ---------------------------------------------------------------------------

def _idx16(tokens, n_slots):
    """[16, n_slots/16] int16: token t at (t%16, t//16). Expanded to 128
    partitions (x8 replication) on device."""
    W = n_slots // 16
    arr = np.zeros((16, W), np.int16)
    t = np.arange(len(tokens))
    arr[t % 16, t // 16] = tokens.astype(np.int16)
    return arr


def _val_tile(vals, n_slots):
    """[128, n_slots/128] fp16: token t at (t%128, t//128)."""
    G = n_slots // 128
    arr = np.zeros((128, G), np.float16)
    t = np.arange(len(vals))
    arr[t % 128, t // 128] = vals.astype(np.float16)
    return arr


def _row_tile(rows, n_slots):
    """[128, n_slots/128] fp16: window-row of token t at (t%128, t//128);
    padded slots get -1 (matches no iota value -> zero S column)."""
    G = n_slots // 128
    arr = np.full((128, G), -1.0, np.float16)
    t = np.arange(len(rows))
    arr[t % 128, t // 128] = rows.astype(np.float16)
    return arr


def _prep_grid_s(per_rank_tokens, n_cells_keys):
    """Grid-S packing: tokens grouped into cells (ordered by key tuple list
    n_cells_keys); each cell padded to max-over-ranks chunk count."""
    nr = len(per_rank_tokens)
    cell_slots = []
    for key in n_cells_keys:
        mx = 0
        for r in range(nr):
            t = per_rank_tokens[r].get(key)
            if t is not None:
                mx = max(mx, len(t[0]))
        cell_slots.append(_pad128(mx))
    total = int(np.sum(cell_slots))
    offs = np.zeros(len(cell_slots) + 1, np.int64)
    offs[1:] = np.cumsum(cell_slots)
    return cell_slots, offs, total


class _Prep:
    pass


def _preprocess(x, xe, gnn_W, cwnn_W0, cwnn_W1, cwnn_W2, Ldo_val, Lup_val,
                edges, row, col, Ldo_idx, Lup_idx):
    p = _Prep()
    f16 = np.float16
    x16 = np.asarray(x, np.float32).astype(f16)
    xe16 = np.asarray(xe, np.float32).astype(f16)
    # per-rank padded shards
    p.x_sh = []
    p.xe_sh = []
    for r in range(NCORES):
        xs = np.zeros((RNP, D), f16)
        xs[:RN] = x16[r * RN:(r + 1) * RN]
        p.x_sh.append(xs)
        xes = np.zeros((REP, D), f16)
        xes[:RE] = xe16[r * RE:(r + 1) * RE]
        p.xe_sh.append(xes)
    p.gnnW = np.ascontiguousarray(
        np.asarray(gnn_W, np.float32).astype(f16).transpose(1, 0, 2).reshape(D, L * D))
    p.W0 = np.ascontiguousarray(
        np.asarray(cwnn_W0, np.float32).astype(f16).transpose(1, 0, 2).reshape(D, L * D))
    p.W1 = np.ascontiguousarray(
        np.asarray(cwnn_W1, np.float32).astype(f16).transpose(1, 0, 2).reshape(D, L * D))
    p.W2 = np.ascontiguousarray(
        np.asarray(cwnn_W2, np.float32).astype(f16).transpose(1, 0, 2).reshape(D, L * D))
    p.ident = np.eye(D, dtype=f16)
    p.iota = np.tile(np.arange(128, dtype=f16)[None, :], (128, 1))
    p.iota256 = np.tile(np.arange(256, dtype=f16)[None, :], (128, 1))

    edges = np.asarray(edges, np.int64)
    row = np.asarray(row, np.int64)
    col = np.asarray(col, np.int64)
    Ldo_idx = np.asarray(Ldo_idx, np.int64)
    Lup_idx = np.asarray(Lup_idx, np.int64)

    # ---------------- GNN tokens (grid-S, 1 bucket, 20 windows) -----------
    src, dst = edges[0], edges[1]
    rk = dst // RN
    per_rank = []
    for r in range(NCORES):
        sel = rk == r
        s_src = src[sel]
        s_src = (s_src // RN) * RNP + s_src % RN
        s_row = dst[sel] - r * RN
        w = s_row // GWIN
        order = np.argsort(w, kind="stable")
        s_src, s_row, w = s_src[order], s_row[order], w[order]
        d = {}
        for wi in range(NWIN_N):
            m = w == wi
            d[wi] = (s_src[m], s_row[m], None)
        per_rank.append(d)
    keys = list(range(NWIN_N))
    cs, offs, Tg = _prep_grid_s(per_rank, keys)
    p.gnn_win_slots = cs          # slots per window (multiple of 128)
    p.gnn_Tg = Tg
    p.gnn_idx = []
    p.gnn_row = []
    for r in range(NCORES):
        toks = np.zeros(Tg, np.int64)
        rows_all = np.full(Tg, -1, np.int64)
        for wi in range(NWIN_N):
            idx, rows, _ = per_rank[r][wi]
            o = offs[wi]
            n = len(idx)
            toks[o:o + n] = idx
            rows_all[o:o + n] = rows - wi * GWIN
        p.gnn_idx.append(_idx16(toks, Tg))
        p.gnn_row.append(_row_tile(rows_all, Tg))

    # ---------------- CWNN tokens (lap x bucket x 256-row window grid) -----
    # token: (col_global, row_local, val) ; lap 0 = Ldo, 1 = Lup
    # Receive side is S-matmul into SBUF agg (no dma_scatter_add): tokens
    # sorted by (lap, bucket, window); each cell padded to max-over-ranks
    # (multiple of 128); duplicates within a chunk are fine (PE sums them).
    lap_data = [(Ldo_idx, np.asarray(Ldo_val)), (Lup_idx, np.asarray(Lup_val))]
    CWWIN = 256
    NWIN_E = REP // CWWIN     # 98
    cell_tokens = [dict() for _ in range(NCORES)]  # key (lap, b, w)
    for lap, (lidx, lval) in enumerate(lap_data):
        lrow, lcol = lidx[0], lidx[1]
        rr = lrow // RE
        for r in range(NCORES):
            sel = rr == r
            c_g = lcol[sel]
            c_g = (c_g // RE) * REP + c_g % RE
            r_l = lrow[sel] - r * RE
            v = lval[sel]
            b = c_g // BUCK
            w = r_l // CWWIN
            key = b * NWIN_E + w
            order = np.argsort(key, kind="stable")
            c_g, r_l, v, key = c_g[order], r_l[order], v[order], key[order]
            for bb in range(NBUCK_E):
                for ww in range(NWIN_E):
                    m = key == bb * NWIN_E + ww
                    if m.any():
                        cell_tokens[r][(lap, bb, ww)] = (
                            c_g[m] - bb * BUCK, r_l[m] - ww * CWWIN, v[m])
    cw_keys = [(lap, bb, ww) for lap in range(2) for bb in range(NBUCK_E)
               for ww in range(NWIN_E)]
    cw_slots = []
    for key in cw_keys:
        mx = 0
        for r in range(NCORES):
            t = cell_tokens[r].get(key)
            if t is not None:
                mx = max(mx, len(t[0]))
        cw_slots.append(_pad128(mx))
    Tc = int(np.sum(cw_slots))
    coffs = np.zeros(len(cw_slots) + 1, np.int64)
    coffs[1:] = np.cumsum(cw_slots)
    p.cw_keys = cw_keys
    p.cw_slots = cw_slots
    p.cw_offs = coffs
    p.cw_Tc = Tc
    p.CWWIN = CWWIN
    p.cw_gidx = []
    p.cw_row = []
    p.cw_val = []
    for r in range(NCORES):
        gt = np.zeros(Tc, np.int64)
        rt = np.full(Tc, -1, np.int64)
        vt = np.zeros(Tc, np.float32)
        for si, key in enumerate(cw_keys):
            t = cell_tokens[r].get(key)
            if t is None:
                continue
            o = coffs[si]
            n = len(t[0])
            gt[o:o + n] = t[0]
            rt[o:o + n] = t[1]
            vt[o:o + n] = t[2]
        p.cw_gidx.append(_idx16(gt, Tc))
        p.cw_row.append(_row_tile(rt, Tc))
        p.cw_val.append(_val_tile(vt, Tc))

    # ---------------- final tokens (bucket x window grid-S) ----------------
    nodes = np.concatenate([row, col])
    cells = np.concatenate([np.arange(E), np.arange(E)])
    rk = nodes // RN
    per_rank_f = []
    for r in range(NCORES):
        sel = rk == r
        cc = cells[sel]
        cc = (cc // RE) * REP + cc % RE
        rl = nodes[sel] - r * RN
        b = cc // BUCK
        w = rl // GWIN
        d = {}
        for bb in range(NBUCK_E):
            for wi in range(NWIN_N):
                m = (b == bb) & (w == wi)
                d[(bb, wi)] = (cc[m] - bb * BUCK, rl[m], None)
        per_rank_f.append(d)
    fkeys = [(bb, wi) for bb in range(NBUCK_E) for wi in range(NWIN_N)]
    fcs, foffs, Tf = _prep_grid_s(per_rank_f, fkeys)
    p.f_keys = fkeys
    p.f_slots = fcs
    p.f_offs = foffs
    p.f_Tf = Tf
    p.f_idx = []
    p.f_row = []
    for r in range(NCORES):
        toks = np.zeros(Tf, np.int64)
        rows_all = np.full(Tf, -1, np.int64)
        for ki, key in enumerate(fkeys):
            idx, rows, _ = per_rank_f[r][key]
            o = foffs[ki]
            n = len(idx)
            toks[o:o + n] = idx
            rows_all[o:o + n] = rows - key[1] * GWIN
        p.f_idx.append(_idx16(toks, Tf))
        p.f_row.append(_row_tile(rows_all, Tf))

    # ---------------- pack everything into one blob per rank --------------
    # (the PJRT/axon path has ~1.6ms per-input-tensor overhead per execute;
    # bytes are nearly free -> ship ONE int16 tensor per rank)
    p.blob_layout = _blob_layout(Tg, Tc, Tf)
    tot = p.blob_layout["_total"]
    p.blob = []
    for r in range(NCORES):
        parts = {
            "x_sh": p.x_sh[r], "xe_sh": p.xe_sh[r],
            "gnnW": p.gnnW, "W0": p.W0, "W1": p.W1, "W2": p.W2,
            "ident": p.ident, "iota": p.iota, "iota256": p.iota256,
            "gnn_idx": p.gnn_idx[r], "gnn_row": p.gnn_row[r],
            "cw_gidx": p.cw_gidx[r], "cw_row": p.cw_row[r],
            "cw_val": p.cw_val[r],
            "f_idx": p.f_idx[r], "f_row": p.f_row[r],
        }
        b = np.empty(tot, np.int16)
        for name, spec in p.blob_layout.items():
            if name == "_total":
                continue
            off, shape, _ = spec
            a = parts[name]
            n = a.size
            b[off:off + n] = np.ascontiguousarray(a).view(np.int16).ravel()
        p.blob.append(b)

    # structure key for compile cache
    p.skey = (Tg, tuple(cs), Tc, tuple(cw_slots), Tf, tuple(fcs))
    return p


def _blob_layout(Tg, Tc, Tf):
    """name -> (element offset, shape, dtype_tag); all 2-byte elements."""
    specs = [
        ("x_sh", (RNP, D), "f16"),
        ("xe_sh", (REP, D), "f16"),
        ("gnnW", (D, L * D), "f16"),
        ("W0", (D, L * D), "f16"),
        ("W1", (D, L * D), "f16"),
        ("W2", (D, L * D), "f16"),
        ("ident", (D, D), "f16"),
        ("iota", (D, D), "f16"),
        ("iota256", (D, 2 * D), "f16"),
        ("gnn_idx", (16, Tg // 16), "i16"),
        ("gnn_row", (128, Tg // 128), "f16"),
        ("cw_gidx", (16, Tc // 16), "i16"),
        ("cw_row", (128, Tc // 128), "f16"),
        ("cw_val", (128, Tc // 128), "f16"),
        ("f_idx", (16, Tf // 16), "i16"),
        ("f_row", (128, Tf // 128), "f16"),
    ]
    layout = {}
    off = 0
    for name, shape, tag in specs:
        n = int(np.prod(shape))
        layout[name] = (off, shape, tag)
        off += n
    layout["_total"] = off
    return layout


# ---------------------------------------------------------------------------
# Device program
# ---------------------------------------------------------------------------

def _build(p, stage=6):
    # stage (cumulative, for HW-time bisection; 6 = full kernel):
    # 0 const loads+out, 1 +table AGs, 2 +CWNN L0 gather/scatter,
    # 3 +W phase L0 + he AG, 4 +GNN L0, 5 +all layers, 6 +final
    import concourse.bacc as bacc
    import concourse.tile as tile
    import concourse.mybir as mybir
    F16 = mybir.dt.float16
    F32 = mybir.dt.float32
    I16 = mybir.dt.int16
    RELU = mybir.ActivationFunctionType.Relu
    COPY = mybir.ActivationFunctionType.Copy
    EQ = mybir.AluOpType.is_equal

    nc = bacc.Bacc("TRN2", target_bir_lowering=False, debug=False,
                   num_devices=NCORES)

    # ---- I/O: ONE packed int16 blob per rank (see _blob_layout) ----
    layout = p.blob_layout
    blob = nc.dram_tensor("blob", [layout["_total"]], I16,
                          kind="ExternalInput")

    def bv(name):
        off, shape, tag = layout[name]
        n = int(np.prod(shape))
        ap = blob.ap()[off:off + n].rearrange("(a b) -> a b", b=shape[1])
        return ap if tag == "i16" else ap.bitcast(F16)

    x_sh_ap = bv("x_sh")
    xe_sh_ap = bv("xe_sh")
    out = nc.dram_tensor("out", [RNP, 2 * D], F16, kind="ExternalOutput")

    # ---- internal DRAM ----
    x16 = nc.dram_tensor("x16_d", [NP_TBL, D], F16, addr_space="Shared")
    xe16 = nc.dram_tensor("xe16_d", [EP_TBL, D], F16, addr_space="Shared")
    h_full = [None,
              nc.dram_tensor("h_full1", [NP_TBL, D], F16, addr_space="Shared"),
              nc.dram_tensor("h_full2", [NP_TBL, D], F16, addr_space="Shared")]
    he_full = [None,
               nc.dram_tensor("he_full1", [EP_TBL, D], F16, addr_space="Shared"),
               nc.dram_tensor("he_full2", [EP_TBL, D], F16, addr_space="Shared"),
               nc.dram_tensor("he_full3", [EP_TBL, D], F16, addr_space="Shared")]
    h_bounce = [nc.dram_tensor(f"h_bounce{i}", [RNP, D], F16) for i in range(2)]
    he_bounce = [nc.dram_tensor(f"he_bounce{i}", [REP, D], F16)
                 for i in range(3)]
    heT_d = [nc.dram_tensor(f"heT{i}", [D, REP], F16) for i in range(2)]
    x_bnc = nc.dram_tensor("x_bnc", [RNP, D], F16)
    xe_bnc = nc.dram_tensor("xe_bnc", [REP, D], F16)

    rg = [list(range(NCORES))]

    with tile.TileContext(nc) as tc:
        with tc.tile_pool(name="const", bufs=1) as cpool, \
             tc.tile_pool(name="gat", bufs=3) as gpool, \
             tc.tile_pool(name="sbld", bufs=2) as spool, \
             tc.tile_pool(name="hTp", bufs=2) as hpool, \
             tc.tile_pool(name="wph", bufs=3) as wpool, \
             tc.tile_pool(name="agg", bufs=2) as apool, \
             tc.tile_pool(name="tt", bufs=4) as tpool, \
             tc.tile_pool(name="ps_s", bufs=2, space="PSUM") as pps, \
             tc.tile_pool(name="ps_c", bufs=4, space="PSUM") as ppc, \
             tc.tile_pool(name="ps_w", bufs=2, space="PSUM") as ppw:

            ident = cpool.tile([D, D], F16, tag="ident")
            nc.sync.dma_start(out=ident[:], in_=bv("ident"))
            iota = cpool.tile([D, D], F16, tag="iota")
            nc.sync.dma_start(out=iota[:], in_=bv("iota"))
            gW = cpool.tile([D, L * D], F16, tag="gW")
            nc.sync.dma_start(out=gW[:], in_=bv("gnnW"))
            w0 = cpool.tile([D, L * D], F16, tag="w0")
            nc.sync.dma_start(out=w0[:], in_=bv("W0"))
            w1 = cpool.tile([D, L * D], F16, tag="w1")
            nc.sync.dma_start(out=w1[:], in_=bv("W1"))
            w2 = cpool.tile([D, L * D], F16, tag="w2")
            nc.sync.dma_start(out=w2[:], in_=bv("W2"))

            # resident index tiles: expand [16, W] -> [128, W] (x8 stripes)
            def expand_idx(src_ap, W, tag):
                t = cpool.tile([128, W], I16, tag=tag)
                for j in range(8):
                    nc.sync.dma_start(out=t[16 * j:16 * (j + 1), :],
                                      in_=src_ap)
                return t

            iota256 = cpool.tile([D, 2 * D], F16, tag="iota256")
            nc.sync.dma_start(out=iota256[:], in_=bv("iota256"))
            gnn_it = expand_idx(bv("gnn_idx"), p.gnn_Tg // 16, "gnn_it")
            cw_git = expand_idx(bv("cw_gidx"), p.cw_Tc // 16, "cw_git")
            f_it = expand_idx(bv("f_idx"), p.f_Tf // 16, "f_it")
            cw_vt = cpool.tile([128, p.cw_Tc // 128], F16, tag="cw_vt")
            nc.sync.dma_start(out=cw_vt[:], in_=bv("cw_val"))
            cw_rt = cpool.tile([128, p.cw_Tc // 128], F16, tag="cw_rt")
            nc.sync.dma_start(out=cw_rt[:], in_=bv("cw_row"))
            gnn_rt = cpool.tile([128, p.gnn_Tg // 128], F16, tag="gnn_rt")
            nc.sync.dma_start(out=gnn_rt[:], in_=bv("gnn_row"))
            f_rt = cpool.tile([128, p.f_Tf // 128], F16, tag="f_rt")
            nc.sync.dma_start(out=f_rt[:], in_=bv("f_row"))

            # SBUF-resident transposed CWNN accumulators [D, REP]
            agg_do = cpool.tile([D, REP], F16, tag="agg_do")
            agg_up = cpool.tile([D, REP], F16, tag="agg_up")

            # transposed local x: hT [D, RNP]
            hT = hpool.tile([D, RNP], F16, tag="hT")
            for wv in range(RNP // WWIN):
                nc.scalar.dma_start_transpose(
                    hT[:, wv * WWIN:(wv + 1) * WWIN],
                    x_sh_ap[wv * WWIN:(wv + 1) * WWIN, :])

            # replicated tables via AllGather (collectives cannot read IO
            # tensors -> bounce through internal DRAM first)
            if stage >= 1:
                nc.sync.dma_start(out=x_bnc.ap(), in_=x_sh_ap)
                for ch in range(4):
                    r0 = ch * (REP // 4)
                    rn = (REP // 4) if ch < 3 else REP - 3 * (REP // 4)
                    nc.sync.dma_start(out=xe_bnc.ap()[r0:r0 + rn, :],
                                      in_=xe_sh_ap[r0:r0 + rn, :])
                nc.gpsimd.collective_compute(
                    "AllGather", mybir.AluOpType.bypass, replica_groups=rg,
                    ins=[x_bnc.ap().opt()], outs=[x16.ap().opt()])
                nc.gpsimd.collective_compute(
                    "AllGather", mybir.AluOpType.bypass, replica_groups=rg,
                    ins=[xe_bnc.ap().opt()], outs=[xe16.ap().opt()])

            # ============ helper: grid-S aggregation ============
            def grid_s_agg(agg, n_win, offs, idx_tile, row_tile, tables,
                           phase):
                """agg: SBUF tile [128, n_win*128]; tables: list of
                (dram_tensor, row_lo, row_n, tok_lo, tok_hi) gather sources."""
                for (src, row_lo, row_n, t0, t1) in tables:
                    t = t0
                    while t < t1:
                        n = min(MAXTOK, t1 - t)
                        G = n // 128
                        g = gpool.tile([128, MAXTOK // 128, D], F16,
                                       tag="g")
                        nc.gpsimd.dma_gather(
                            g[:, :G, :], src.ap()[row_lo:row_lo + row_n, :],
                            idx_tile[:, t // 16:(t + n) // 16], n, n, D)
                        # one batched S build for all G chunks of this gather
                        stb = spool.tile([128, MAXTOK // 128, GWIN], F16,
                                         tag="s1h")
                        nc.vector.tensor_tensor(
                            out=stb[:, :G, :],
                            in0=iota[:].unsqueeze(1)
                                .broadcast_to([128, G, GWIN]),
                            in1=row_tile[:, t // 128:(t + n) // 128]
                                .unsqueeze(2).broadcast_to([128, G, GWIN]),
                            op=EQ)
                        for c in range(G):
                            tok = t + c * 128
                            ki = int(np.searchsorted(offs, tok, side="right")) - 1
                            wi = ki % n_win if phase == "f" else ki
                            ps = pps.tile([128, GWIN], F32, tag="ps_s")
                            nc.tensor.matmul(
                                ps[:], lhsT=g[:, c, :], rhs=stb[:, c, :],
                                start=True, stop=True)
                            nc.vector.tensor_tensor(
                                out=agg[:, wi * GWIN:(wi + 1) * GWIN],
                                in0=agg[:, wi * GWIN:(wi + 1) * GWIN],
                                in1=ps[:], op=mybir.AluOpType.add)
                        t += n

            # ============ interleaved layers ============
            n_layers = L if stage >= 5 else (1 if stage >= 2 else 0)
            CWWIN = p.CWWIN
            for i in range(n_layers):
                # ---------- CWNN layer i ----------
                he_table = xe16 if i == 0 else he_full[i]
                heT_nxt = heT_d[i % 2]

                nc.vector.memset(agg_do[:], 0.0)
                nc.vector.memset(agg_up[:], 0.0)

                # gather ranges are contiguous per (lap, bucket); chunks map
                # statically to (lap, b, window) cells via cw_offs
                coffs = p.cw_offs
                lb_ranges = []
                for lap in range(2):
                    for bb in range(NBUCK_E):
                        k0 = (lap * NBUCK_E + bb) * (REP // CWWIN)
                        k1 = k0 + (REP // CWWIN)
                        t0, t1 = int(coffs[k0]), int(coffs[k1])
                        if t1 > t0:
                            lb_ranges.append((bb, t0, t1))
                for (bb, t0, t1) in lb_ranges:
                    row_lo = bb * BUCK
                    row_n = min(BUCK, EP_TBL - row_lo)
                    t = t0
                    while t < t1:
                        n = min(MAXTOK, t1 - t)
                        G = n // 128
                        g = gpool.tile([128, MAXTOK // 128, D], F16, tag="g")
                        nc.gpsimd.dma_gather(
                            g[:, :G, :], he_table.ap()[row_lo:row_lo + row_n, :],
                            cw_git[:, t // 16:(t + n) // 16], n, n, D)
                        nc.vector.tensor_tensor(
                            out=g[:, :G, :], in0=g[:, :G, :],
                            in1=cw_vt[:, t // 128:(t + n) // 128]
                                .unsqueeze(2).broadcast_to([128, G, D]),
                            op=mybir.AluOpType.mult)
                        # one batched S build for all G chunks of this gather
                        stb = spool.tile([128, MAXTOK // 128, CWWIN], F16,
                                         tag="s2h")
                        nc.vector.tensor_tensor(
                            out=stb[:, :G, :],
                            in0=iota256[:].unsqueeze(1)
                                .broadcast_to([128, G, CWWIN]),
                            in1=cw_rt[:, t // 128:(t + n) // 128]
                                .unsqueeze(2).broadcast_to([128, G, CWWIN]),
                            op=EQ)
                        for c in range(G):
                            tok = t + c * 128
                            ki = int(np.searchsorted(coffs, tok,
                                                     side="right")) - 1
                            lap, _, ww = p.cw_keys[ki]
                            agg = agg_do if lap == 0 else agg_up
                            sl = agg[:, ww * CWWIN:(ww + 1) * CWWIN]
                            ps = ppc.tile([128, CWWIN], F32, tag="ps_c")
                            # agg[ww] += msgs^T @ S, all on PE/Act (no DVE
                            # PSUM reads): fold old agg in via ident matmul,
                            # drain PSUM back to SBUF on the Act engine
                            nc.tensor.matmul(ps[:], lhsT=g[:, c, :],
                                             rhs=stb[:, c, :],
                                             start=True, stop=False)
                            nc.tensor.matmul(ps[:], lhsT=ident[:], rhs=sl,
                                             start=False, stop=True)
                            nc.scalar.activation(sl, ps[:], COPY)
                        t += n

                if stage < 3:
                    break
                # W phase: he_next = relu(he@W0 + acc_do@W1 + acc_up@W2)
                # (agg_do/agg_up already live in SBUF, transposed)
                for w in range(REP // WWIN):
                    c0 = w * WWIN
                    cn = WWIN
                    hw = wpool.tile([128, WWIN], F16, tag="hw")
                    if i == 0:
                        nc.scalar.dma_start_transpose(
                            hw[:, :cn], xe_sh_ap[c0:c0 + cn, :])
                    else:
                        nc.sync.dma_start(
                            out=hw[:, :cn],
                            in_=heT_d[(i + 1) % 2].ap()[:, c0:c0 + cn])
                    ps = ppw.tile([128, WWIN], F32, tag="ps_w")
                    nc.tensor.matmul(ps[:, :cn], lhsT=w0[:, i * D:(i + 1) * D],
                                     rhs=hw[:, :cn], start=True, stop=False)
                    nc.tensor.matmul(ps[:, :cn], lhsT=w1[:, i * D:(i + 1) * D],
                                     rhs=agg_do[:, c0:c0 + cn],
                                     start=False, stop=False)
                    nc.tensor.matmul(ps[:, :cn], lhsT=w2[:, i * D:(i + 1) * D],
                                     rhs=agg_up[:, c0:c0 + cn],
                                     start=False, stop=True)
                    hn = wpool.tile([128, WWIN], F16, tag="hn")
                    nc.scalar.activation(hn[:, :cn], ps[:, :cn], RELU)
                    nc.sync.dma_start(out=heT_nxt.ap()[:, c0:c0 + cn],
                                      in_=hn[:, :cn])
                    # row-major blocks for AllGather input via PE transpose
                    rows = tpool.tile([128, WWIN // 128, D], F16, tag="cw_rr")
                    for tt_i in range(WWIN // 128):
                        r0 = tt_i * 128
                        pst = pps.tile([128, D], F32, tag="ps_s")
                        nc.tensor.matmul(pst[:], lhsT=hn[:, r0:r0 + 128],
                                         rhs=ident[:], start=True, stop=True)
                        nc.scalar.activation(rows[:, tt_i, :], pst[:], COPY)
                    nc.sync.dma_start(
                        out=he_bounce[i].ap()[c0:c0 + cn, :]
                            .rearrange("(g q) d -> q g d", q=128),
                        in_=rows[:])

                nc.gpsimd.collective_compute(
                    "AllGather", mybir.AluOpType.bypass, replica_groups=rg,
                    ins=[he_bounce[i].ap().opt()],
                    outs=[he_full[i + 1].ap().opt()])

                if stage < 4:
                    break
                # ---------- GNN layer i ----------
                h_table = x16 if i == 0 else h_full[i]
                agg = apool.tile([128, RNP], F16, tag="gagg")
                nc.vector.memset(agg[:], 0.0)
                grid_s_agg(agg, NWIN_N, p.gnn_win_offs_np, gnn_it, gnn_rt,
                           [(h_table, 0, NP_TBL, 0, p.gnn_Tg)], "g")
                nc.vector.tensor_tensor(out=agg[:], in0=agg[:],
                                        in1=hT[:],
                                        op=mybir.AluOpType.add)
                hT = hpool.tile([D, RNP], F16, tag="hT")
                for w in range(_ceil(RNP, WWIN)):
                    c0 = w * WWIN
                    cn = min(WWIN, RNP - c0)
                    ps = ppw.tile([128, WWIN], F32, tag="ps_w")
                    nc.tensor.matmul(ps[:, :cn], lhsT=gW[:, i * D:(i + 1) * D],
                                     rhs=agg[:, c0:c0 + cn],
                                     start=True, stop=True)
                    nc.scalar.activation(hT[:, c0:c0 + cn], ps[:, :cn], RELU)
                if i < L - 1:
                    for t in range(RNP // 128):
                        r0 = t * 128
                        tt = tpool.tile([128, 128], F16, tag="g_tt")
                        nc.scalar.dma_start_transpose(
                            tt[:], hT[:, r0:r0 + 128])
                        nc.sync.dma_start(
                            out=h_bounce[i].ap()[r0:r0 + 128, :],
                            in_=tt[:])
                    nc.gpsimd.collective_compute(
                        "AllGather", mybir.AluOpType.bypass, replica_groups=rg,
                        ins=[h_bounce[i].ap().opt()],
                        outs=[h_full[i + 1].ap().opt()])

            # ============ final: xed = segsum(he3, row) + segsum(he3, col) ==
            fagg = apool.tile([128, RNP], F16, tag="fagg")
            nc.vector.memset(fagg[:], 0.0)
            ftables = []
            for bi, bb in enumerate(range(NBUCK_E) if stage >= 6 else []):
                klo = bi * NWIN_N
                t0 = int(p.f_offs[klo])
                t1 = int(p.f_offs[klo + NWIN_N])
                row_lo = bb * BUCK
                row_n = min(BUCK, EP_TBL - row_lo)
                if t1 > t0:
                    ftables.append((he_full[3], row_lo, row_n, t0, t1))
            grid_s_agg(fagg, NWIN_N, p.f_offs, f_it, f_rt, ftables, "f")

            # output: [RNP, 0:128] = h3 rows, [RNP, 128:256] = xed
            for t in range(NWIN_N):
                c0 = t * 128
                psx = pps.tile([128, 128], F32, tag="ps_s")
                nc.tensor.matmul(psx[:], lhsT=hT[:, c0:c0 + 128], rhs=ident[:],
                                 start=True, stop=True)
                ox = tpool.tile([128, 128], F16, tag="tt16")
                nc.vector.tensor_copy(ox[:], psx[:])
                nc.sync.dma_start(out=out.ap()[c0:c0 + 128, 0:D], in_=ox[:])
                psy = pps.tile([128, 128], F32, tag="ps_s")
                nc.tensor.matmul(psy[:], lhsT=fagg[:, c0:c0 + 128],
                                 rhs=ident[:], start=True, stop=True)
                oy = tpool.tile([128, 128], F16, tag="tt16")
                nc.vector.tensor_copy(oy[:], psy[:])
                nc.sync.dma_start(out=out.ap()[c0:c0 + 128, D:2 * D], in_=oy[:])

    nc.compile()
    return nc


# ---------------------------------------------------------------------------
# PJRT runner (axon path; no /dev/neuron* on client)
# ---------------------------------------------------------------------------

def _make_runner(nc):
    import jax
    import time
    from jax.sharding import Mesh, PartitionSpec
    from jax.experimental.shard_map import shard_map
    import concourse.mybir as mybir
    import concourse.bass2jax as bass2jax
    from concourse.bass2jax import _bass_exec_p, install_neuronx_cc_hook

    install_neuronx_cc_hook()
    partition_name = nc.partition_id_tensor.name if nc.partition_id_tensor else None

    in_names, out_names, out_avals, zero_outs = [], [], [], []
    for alloc in nc.m.functions[0].allocations:
        if not isinstance(alloc, mybir.MemoryLocationSet):
            continue
        name = alloc.memorylocations[0].name
        if alloc.kind == "ExternalInput":
            if name != partition_name:
                in_names.append(name)
        elif alloc.kind == "ExternalOutput":
            out_names.append(name)
            shape = tuple(alloc.tensor_shape)
            dtype = mybir.dt.np(alloc.dtype)
            out_avals.append(jax.core.ShapedArray(shape, dtype))
            zero_outs.append(np.zeros(shape, dtype))
    n_params = len(in_names)
    all_in_names = list(in_names) + list(out_names)
    if partition_name is not None:
        all_in_names.append(partition_name)

    def _body(*args):
        operands = list(args)
        if partition_name is not None:
            operands.append(bass2jax.partition_id_tensor())
        outs = _bass_exec_p.bind(
            *operands,
            out_avals=tuple(out_avals),
            in_names=tuple(all_in_names),
            out_names=tuple(out_names),
            lowering_input_output_aliases=(),
            sim_require_finite=True,
            sim_require_nnan=True,
            nc=nc,
        )
        return tuple(outs)

    devices = jax.devices()[:NCORES]
    mesh = Mesh(np.asarray(devices), ("core",))
    in_specs = (PartitionSpec("core"),) * (n_params + len(out_names))
    out_specs = (PartitionSpec("core"),) * len(out_names)
    sharded = jax.jit(
        shard_map(_body, mesh=mesh, in_specs=in_specs, out_specs=out_specs,
                  check_rep=False),
        keep_unused=True,
    )

    def run_fn(in_maps, iters=1):
        per_core = [[np.asarray(m[name]) for name in in_names] for m in in_maps]
        concat_in = [np.concatenate([per_core[c][i] for c in range(NCORES)], axis=0)
                     for i in range(n_params)]
        concat_zeros = [np.zeros((NCORES * z.shape[0], *z.shape[1:]), z.dtype)
                        for z in zero_outs]
        dev_in = [jax.device_put(a) for a in concat_in]
        dev_zero = [jax.device_put(z) for z in concat_zeros]
        out = sharded(*dev_in, *dev_zero)
        jax.block_until_ready(out)
        t0 = time.perf_counter()
        if iters > 1:
            for _ in range(iters):
                out = sharded(*dev_in, *dev_zero)
            jax.block_until_ready(out)
            dt = (time.perf_counter() - t0) / iters
        else:
            dt = 0.0
        results = [
            {name: np.asarray(out[i]).reshape(NCORES, *out_avals[i].shape)[c]
             for i, name in enumerate(out_names)}
            for c in range(NCORES)
        ]
        return results, dt

    return run_fn

# ---------------------------------------------------------------------------

_CACHE = {}


def _get_runner(p):
    key = p.skey
    if key in _CACHE:
        return _CACHE[key]
    # np arrays needed by builder
    offs = np.zeros(NWIN_N + 1, np.int64)
    offs[1:] = np.cumsum(p.gnn_win_slots)
    p.gnn_win_offs_np = offs
    nc = _build(p)
    run_fn = _make_runner(nc)
    _CACHE[key] = run_fn
    return run_fn


def kernel(**inputs):
    p = _preprocess(**inputs)
    run_fn = _get_runner(p)
    in_maps = [{"blob": p.blob[r]} for r in range(NCORES)]
    results, dt = run_fn(in_maps, iters=1)
    kernel.last_dt = dt
    kernel.run_fn = run_fn
    kernel.in_maps = in_maps
    outs = [results[r]["out"][:RN] for r in range(NCORES)]
    return np.concatenate(outs, axis=0).astype(np.float32)
